# revision 58
# baseline (speedup 1.0000x reference)
"""Trainium2 Bass kernel for causal multi-head attention (B=4, T=2048, D=1024, H=16).

Sharding: 8 cores = 4 batches x 2 head-groups (8 heads each).
Per core pipeline (Tile framework, single SPMD program):
  phase 1(j): Q/K projections into transposed per-head-pair layout QT/KT [128=2*64, T],
           V projection into [t, 8*65] layout (65th col per head = ones, for rowsums)
  phase 2(j): per (q-range of 512, head-pair): causal flash attention in transposed
           layout: ST[k,q] = KT-slice^T @ QT-slice (row-packed pair of matmuls),
           PT = exp(ST) (ACT), causal triangle mask on diagonal 128-col strips (DVE),
           OT[hd+1, q] += [V|1]^T @ PT (bf16), normalize by approx-reciprocal rowsum.
  phase 3(j): output projection YT[dout, t] = Wo_gT^T @ OT, DMA'd straight from PSUM.
Phase 1(j+1) matmul chunks are emitted interleaved into phase 2(j) so the PE fills
its exp-wait gaps with projection work (phase 2 is ACT-bound; phases 1/3 PE-bound).
No collective: each core emits its partial YT [D, T]; the host adds the two partial
sums of each batch pair and adds the output bias.
"""

import numpy as np

B, T, D, H, HD = 4, 2048, 1024, 16, 64
NCORES = 8
NP = 4          # head pairs per core
NJ = 4          # q-ranges of 512
QW = 512
TB = T // 128   # 16

_CACHE = {}


def _build_nc():
    import concourse.mybir as mybir
    import concourse.tile as tile
    from concourse import bacc

    F32 = mybir.dt.float32
    BF16 = mybir.dt.bfloat16
    FP8 = mybir.dt.float8e4
    AF = mybir.ActivationFunctionType
    DR = mybir.MatmulPerfMode.DoubleRow

    nc = bacc.Bacc(None, target_bir_lowering=False)

    # Pin all activations to the one table holding Exp+Ln+Copy so the
    # act-table chooser can't thrash loads between the exp stream and the
    # exp(-ln(x)) reciprocal. Indices must match act_info.json, so other
    # tables are emptied rather than removed.
    import types as _types
    from concourse.hw_specs import get_activation_tables as _gat

    def _pinned_act_table_loads(self):
        import bass_rust as _bass_rust
        import concourse.mybir as _mybir
        has_activation = any(
            isinstance(i, _mybir.InstActivation)
            for b in self.main_func.blocks
            for i in b.instructions
        )
        if not has_activation:
            return
        tables = [
            (name, funcs if name == "natural_log_exp_and_others" else set())
            for name, funcs in _gat(self.m.arch).items()
        ]
        _bass_rust.insert_act_table_loads(self, tables)

    nc.insert_act_table_loads = _types.MethodType(_pinned_act_table_loads, nc)

    xt8_d = nc.declare_dram_parameter("xt8", [NJ, 128, 8 * QW], FP8, isOutput=False)
    xt_d = nc.declare_dram_parameter("xt", [NJ, 128, 8 * QW], BF16, isOutput=False)
    wq_d = nc.declare_dram_parameter("wq", [128, 8 * 512], FP8, isOutput=False)
    wk_d = nc.declare_dram_parameter("wk", [128, 8 * 512], FP8, isOutput=False)
    wv_d = nc.declare_dram_parameter("wv", [128, 8 * 512], BF16, isOutput=False)
    wo_d = nc.declare_dram_parameter("wo", [128, 4 * D], BF16, isOutput=False)
    mask_d = nc.declare_dram_parameter("mask", [128, 128], BF16, isOutput=False)
    yt_d = nc.declare_dram_parameter("yt", [D, T], F32, isOutput=True)

    with tile.TileContext(nc) as tc:
        with (
            tc.tile_pool(name="persist", bufs=1) as pers,
            tc.tile_pool(name="work", bufs=1) as work,
            tc.tile_pool(name="psum", bufs=1, space="PSUM") as psum,
        ):
            qt = pers.tile([128, NP, T], BF16)
            kt = pers.tile([128, NP, T], BF16)
            v = pers.tile([128, TB, 8 * 65], BF16)
            ot = pers.tile([128, NP, T], BF16)
            m0 = pers.tile([128, 128], BF16)
            wo = pers.tile([128, 4, D], BF16)
            wq = pers.tile([128, 8, 512], FP8)
            wk = pers.tile([128, 8, 512], FP8)
            wv = pers.tile([128, 8, 512], BF16)

            xs_tiles = {}
            xsb_tiles = {}

            def load_xs(j):
                t = work.tile([128, 8, QW], FP8, tag="xs", bufs=3)
                tb = work.tile([128, 8, QW], BF16, tag="xsb", bufs=3)
                nc.sync.dma_start(
                    out=t[:], in_=xt8_d[j].rearrange("p (c n) -> p c n", c=8)
                )
                nc.sync.dma_start(
                    out=tb[:], in_=xt_d[j].rearrange("p (c n) -> p c n", c=8)
                )
                xs_tiles[j] = t
                xsb_tiles[j] = tb

            # startup DMAs: weights are host-prearranged [128, ...] so each is
            # one fully contiguous transfer per partition
            nc.sync.dma_start(out=m0[:], in_=mask_d[:])
            nc.sync.dma_start(out=wq[:], in_=wq_d.rearrange("p (c n) -> p c n", c=8))
            nc.sync.dma_start(out=wk[:], in_=wk_d.rearrange("p (c n) -> p c n", c=8))
            load_xs(0)
            nc.sync.dma_start(out=wv[:], in_=wv_d.rearrange("p (c n) -> p c n", c=8))
            nc.sync.dma_start(out=wo[:], in_=wo_d.rearrange("p (c n) -> p c n", c=4))

            # Filler work is emitted as ~2-matmul micro-chunks: a whole 8-MM
            # chunk in the PE FIFO delays the next ST by up to ~1.8us, which
            # starves the ACT exp stream (the phase-2 critical path). Each
            # chunk is a pair of closures sharing one PSUM accumulator; the
            # pair stays adjacent in the drain list so at most two
            # accumulation groups are ever open on the "small" tag.
            # Weights are pre-scaled x128 on the host so they sit in e4m3's
            # normal range; the 1/128 is undone in the PSUM->SBUF copy.
            def qk_micros(j, p, w_sb, dst):
                st_ = {}

                def half(h):
                    if h == 0:
                        acc = psum.tile([128, QW], F32, tag="small", bufs=2)
                        st_["acc"] = acc
                    acc = st_["acc"]
                    for cp in (2 * h, 2 * h + 1):
                        nc.tensor.matmul(
                            acc[:],
                            w_sb[:, 2 * cp:2 * cp + 2, p * 128:(p + 1) * 128],
                            xs_tiles[j][:, 2 * cp:2 * cp + 2, :],
                            start=(cp == 0),
                            stop=(cp == 3),
                            perf_mode=DR,
                        )
                    if h == 1:
                        nc.vector.tensor_scalar_mul(
                            dst[:, p, j * QW:(j + 1) * QW], acc[:], 1.0 / 128.0
                        )
                return [lambda: half(0), lambda: half(1)]

            def v_micros(j, sub):
                # V path stays bf16: early tokens average few keys, so v
                # quantization error doesn't wash out like q/k error does
                i = 4 * j + sub
                st_ = {}

                def half(h):
                    if h == 0:
                        acc = psum.tile([128, QW], F32, tag="small", bufs=2)
                        st_["acc"] = acc
                    acc = st_["acc"]
                    for c in range(4 * h, 4 * h + 4):
                        nc.tensor.matmul(
                            acc[:],
                            xsb_tiles[j][:, c, sub * 128:(sub + 1) * 128],
                            wv[:, c, :],
                            start=(c == 0),
                            stop=(c == 7),
                        )
                    if h == 1:
                        vblk = v[:, i, :].rearrange("p (h c) -> p h c", c=65)
                        nc.vector.tensor_copy(
                            vblk[:, :, 0:64],
                            acc[:].rearrange("p (h c) -> p h c", c=64),
                        )
                        nc.gpsimd.memset(vblk[:, :, 64:65], 1.0)
                return [lambda: half(0), lambda: half(1)]

            def phase1_micros(j, v_early=False):
                ch = []
                for p in range(NP):
                    ch += qk_micros(j, p, wq, qt)
                    ch += qk_micros(j, p, wk, kt)
                    if v_early and p == 0:
                        for sub in range(4):
                            ch += v_micros(j, sub)
                if not v_early:
                    for sub in range(4):
                        ch += v_micros(j, sub)
                return ch

            # phase 1(0): only Q0/K0 up front — everything else interleaves
            # into phase 2(0)'s slots so the exp stream starts ~10us earlier
            for m_ in qk_micros(0, 0, wq, qt) + qk_micros(0, 0, wk, kt):
                m_()
            load_xs(1)
            # V blocks land just before their AVs; Q/K(p) just before pair p.
            # Targets are non-decreasing along the list (drain pops in order).
            leftover0 = (
                v_micros(0, 0)
                + qk_micros(0, 1, wq, qt) + v_micros(0, 1)
                + qk_micros(0, 1, wk, kt) + v_micros(0, 2)
                + v_micros(0, 3)
                + qk_micros(0, 2, wq, qt) + qk_micros(0, 2, wk, kt)
                + qk_micros(0, 3, wq, qt) + qk_micros(0, 3, wk, kt)
            )
            leftover0_targets = [0, 0, 1, 1, 1, 1, 2, 2, 2, 2,
                                 3, 3, 4, 4, 5, 5, 8, 8, 9, 9]

            def p3_micros(j, n):
                jrp = slice(j * QW, (j + 1) * QW)
                st_ = {}

                def half(h):
                    if h == 0:
                        yps = psum.tile([128, QW], F32, tag="small", bufs=2)
                        st_["y"] = yps
                    yps = st_["y"]
                    for c4 in (2 * h, 2 * h + 1):
                        nc.tensor.matmul(
                            yps[:],
                            wo[:, c4, n * 128:(n + 1) * 128],
                            ot[:, c4, jrp],
                            start=(c4 == 0), stop=(c4 == 3),
                        )
                    if h == 1:
                        ysb = work.tile([128, QW], F32, tag="ysb", bufs=3)
                        nc.vector.tensor_copy(ysb[:], yps[:])
                        nc.sync.dma_start(
                            out=yt_d[n * 128:(n + 1) * 128, jrp], in_=ysb[:]
                        )
                return [lambda: half(0), lambda: half(1)]

            def phase3_chunks(j):
                ch = []
                for n in range(8):
                    ch += p3_micros(j, n)
                return ch

            carry_kv = []
            for j in range(NJ):
                jr = slice(j * QW, (j + 1) * QW)
                # PE filler work for this j's ACT-bound attention stream:
                # previous j's output projection + next j's projections
                p3 = phase3_chunks(j - 1) if j > 0 else []
                if j + 2 < NJ:
                    load_xs(j + 2)
                nkb = 4 * j + 4
                slots = NP * nkb
                if j + 1 == NJ - 1:
                    # defer the last j's K/V projections into phase 2(3)
                    # itself: only its diagonal blocks (kb>=12) consume them,
                    # and j=3 is the ACT-bound stretch that starves the PE
                    p1 = []
                    for p in range(NP):
                        p1 += qk_micros(j + 1, p, wq, qt)
                    carry_kv = []
                    for p in range(NP):
                        carry_kv += qk_micros(j + 1, p, wk, kt)
                    for s in range(4):
                        carry_kv += v_micros(j + 1, s)
                elif j + 1 < NJ:
                    p1 = phase1_micros(j + 1)
                else:
                    p1 = []
                if j == NJ - 1:
                    # deferred K/V first (needed from kb=12 on), then the
                    # previous j's output projection spread across the rest
                    chunks = carry_kv + p3
                    targets = [1 + (i * 10) // len(carry_kv) for i in range(len(carry_kv))] + [
                        12 + (3 * i) for i in range(len(p3))
                    ]
                else:
                    # interleave micro-pairs of phase3(j-1) and phase1(j+1),
                    # keeping each chunk's two halves adjacent
                    chunks = []
                    for i in range(max(len(p3), len(p1)) // 2 + 1):
                        if 2 * i < len(p3):
                            chunks += p3[2 * i:2 * i + 2]
                        if 2 * i < len(p1):
                            chunks += p1[2 * i:2 * i + 2]
                    targets = [
                        (i + 1) * slots // (len(chunks) + 3)
                        for i in range(len(chunks))
                    ]
                    if j == 0:
                        # rest of phase 1(0), placed just ahead of its consumers
                        chunks = leftover0 + chunks
                        targets = leftover0_targets + [
                            max(t, 13) for t in targets
                        ]
                emitted = 0
                slot = 0
                ocps = []

                def emit_norm(p):
                    # 1/r = exp(-ln(r)): Ln/Exp share the pinned act table.
                    # Emitted one head-pair late so the ln never waits on the
                    # ocp copy inside the strict ACT FIFO.
                    lnr = work.tile([1, 1024], F32, tag="lnr", bufs=2)
                    nc.scalar.activation(lnr[:], ocps[p][64:65, :], AF.Ln)
                    rec = work.tile([1, 1024], F32, tag="rec", bufs=4)
                    nc.scalar.activation(rec[:], lnr[:], AF.Exp, scale=-1.0)
                    bc = work.tile([64, 1024], F32, tag="bc", bufs=3)
                    nc.gpsimd.partition_broadcast(bc[:, 0:QW], rec[:, 0:QW], channels=64)
                    nc.gpsimd.partition_broadcast(bc[:, QW:1024], rec[:, QW:1024], channels=64)
                    nc.vector.tensor_mul(ot[0:64, p, jr], ocps[p][0:64, 0:QW], bc[:, 0:QW])
                    nc.vector.tensor_mul(ot[64:128, p, jr], ocps[p][0:64, QW:1024], bc[:, QW:1024])

                # ---------------- phase 2(j) with phase 1(j+1) interleaved ----------
                for p in range(NP):
                    hA, hB = 2 * p, 2 * p + 1
                    o_A = psum.tile([65, QW], F32, tag="o", bufs=2)
                    o_B = psum.tile([65, QW], F32, tag="o", bufs=2)
                    for kb in range(nkb):
                        o = kb - 4 * j  # diagonal offset; < 0 means full block
                        lo = 128 * o if o > 0 else 0
                        st = psum.tile([128, 1024], F32, tag="st", bufs=2)
                        kcols = slice(kb * 128, (kb + 1) * 128)
                        qcols = slice(j * QW + lo, (j + 1) * QW)
                        nc.tensor.matmul(
                            st[:, lo:QW],
                            kt[0:64, p, kcols],
                            qt[0:64, p, qcols],
                            start=True, stop=True, tile_position=(0, 0),
                        )
                        nc.tensor.matmul(
                            st[:, QW + lo:2 * QW],
                            kt[64:128, p, kcols],
                            qt[64:128, p, qcols],
                            start=True, stop=True, tile_position=(64, 0),
                        )
                        pt = work.tile([128, 1024], BF16, tag="pt", bufs=3)
                        nc.scalar.activation(
                            pt[:].rearrange("p (h q) -> p h q", h=2)[:, :, lo:QW],
                            st[:].rearrange("p (h q) -> p h q", h=2)[:, :, lo:QW],
                            AF.Exp,
                        )
                        if o >= 0:
                            # only the leading 128-col strip of the valid range
                            # holds the causal triangle
                            nc.vector.tensor_mul(
                                pt[:, lo:lo + 128], pt[:, lo:lo + 128], m0[:]
                            )
                            nc.vector.tensor_mul(
                                pt[:, QW + lo:QW + lo + 128],
                                pt[:, QW + lo:QW + lo + 128],
                                m0[:],
                            )
                        # drain filler here: these matmuls sit in the PE
                        # FIFO between the STs and the AVs, executing inside
                        # the exp's latency window the AVs must wait out
                        while emitted < len(chunks) and slot >= targets[emitted]:
                            chunks[emitted]()
                            emitted += 1
                        nc.tensor.matmul(
                            o_A[:, lo:QW],
                            v[:, kb, hA * 65:(hA + 1) * 65],
                            pt[:, lo:QW],
                            start=(kb == 0), stop=(kb == nkb - 1),
                        )
                        nc.tensor.matmul(
                            o_B[:, lo:QW],
                            v[:, kb, hB * 65:(hB + 1) * 65],
                            pt[:, QW + lo:2 * QW],
                            start=(kb == 0), stop=(kb == nkb - 1),
                        )
                        slot += 1
                    # stage o out of PSUM promptly so the o slots free for the
                    # next head-pair (keeps PE from stalling / HAM warm)
                    ocp = work.tile([65, 1024], F32, tag="ocp", bufs=5)
                    nc.vector.tensor_copy(ocp[:, 0:QW], o_A[:])
                    nc.vector.tensor_copy(ocp[:, QW:1024], o_B[:])
                    ocps.append(ocp)
                    if p >= 1:
                        emit_norm(p - 1)
                emit_norm(NP - 1)
                while emitted < len(chunks):
                    chunks[emitted]()
                    emitted += 1

            # last j's output projection (the tail): pipeline two n-blocks so
            # the c=0..2 accumulations run while the final normalize drains
            jrp = slice((NJ - 1) * QW, NJ * QW)
            for npair in range(4):
                yy = []
                for n in (2 * npair, 2 * npair + 1):
                    yps = psum.tile([128, QW], F32, tag="small", bufs=2)
                    for c4 in range(3):
                        nc.tensor.matmul(
                            yps[:],
                            wo[:, c4, n * 128:(n + 1) * 128],
                            ot[:, c4, jrp],
                            start=(c4 == 0), stop=False,
                        )
                    yy.append(yps)
                for i, n in enumerate((2 * npair, 2 * npair + 1)):
                    yps = yy[i]
                    nc.tensor.matmul(
                        yps[:],
                        wo[:, 3, n * 128:(n + 1) * 128],
                        ot[:, 3, jrp],
                        start=False, stop=True,
                    )
                    ysb = work.tile([128, QW], F32, tag="ysb", bufs=3)
                    nc.vector.tensor_copy(ysb[:], yps[:])
                    nc.sync.dma_start(
                        out=yt_d[n * 128:(n + 1) * 128, jrp], in_=ysb[:]
                    )

    nc.finalize()
    return nc


def _prep_inputs(x, Wq, Wk, Wv, Wo, bo):
    """Build the 8 per-core input maps (host-side layout prep only)."""
    import ml_dtypes

    scale = 1.0 / np.sqrt(np.float32(HD))
    kr = np.arange(128, dtype=np.float32)[:, None]
    qc = np.arange(128, dtype=np.float32)[None, :]
    m0 = (qc >= kr).astype(ml_dtypes.bfloat16)

    FP8 = ml_dtypes.float8_e4m3  # TRN FP8_EXP4-compatible for |x| <= 240

    def xarr(xb, dtype):  # [T, D] -> [NJ, 128, 8*512], one contiguous DMA per j
        xt = xb.T  # [D, T]
        out = np.stack(
            [
                xt[:, j * QW:(j + 1) * QW]
                .reshape(8, 128, QW).transpose(1, 0, 2).reshape(128, 8 * QW)
                for j in range(NJ)
            ]
        )
        return np.ascontiguousarray(out).astype(dtype)

    xt8s = [xarr(np.clip(x[b], -240, 240), FP8) for b in range(B)]
    xts = [xarr(x[b], ml_dtypes.bfloat16) for b in range(B)]
    in_maps = []
    for c in range(NCORES):
        b, g = c // 2, c % 2
        hs = slice(g * 8, (g + 1) * 8)
        # x128 prescale keeps the small weights inside e4m3's normal range;
        # the kernel multiplies the projection PSUM by 1/128 when casting out.
        # layouts are [128, c*...] so each weight loads as one contiguous DMA
        def warr(wt, dtype):  # [D, 512] -> [128, 8*512], row p = concat_c w[c*128+p]
            return np.ascontiguousarray(
                wt.reshape(8, 128, 512).transpose(1, 0, 2).reshape(128, 8 * 512)
            ).astype(dtype)

        wqc = warr(Wq[hs].reshape(512, D).T * (scale * 128), FP8)
        wkc = warr(Wk[hs].reshape(512, D).T * 128, FP8)
        wvc = warr(Wv[hs].reshape(512, D).T, ml_dtypes.bfloat16)
        woc = np.ascontiguousarray(
            Wo[:, g * 512:(g + 1) * 512].T.reshape(4, 128, D).transpose(1, 0, 2).reshape(128, 4 * D)
        ).astype(ml_dtypes.bfloat16)
        in_maps.append(
            {"xt8": xt8s[b], "xt": xts[b], "wq": wqc, "wk": wkc, "wv": wvc,
             "wo": woc, "mask": m0}
        )
    return in_maps


def _assemble(yts, bo):
    """Sum the per-core partial outputs of each batch pair, add bias."""
    y = np.empty((B, T, D), np.float32)
    for b in range(B):
        y[b] = (yts[2 * b] + yts[2 * b + 1]).T
    y += bo.astype(np.float32)[None, None, :]
    return y


def _run(inputs, trace=False, trace_cores=None):
    from concourse.bass_utils import run_bass_kernel_spmd

    if "nc" not in _CACHE:
        _CACHE["nc"] = _build_nc()
    nc = _CACHE["nc"]
    in_maps = _prep_inputs(
        inputs["x"], inputs["Wq"], inputs["Wk"], inputs["Wv"], inputs["Wo"], inputs["bo"]
    )
    r = run_bass_kernel_spmd(
        nc, in_maps, list(range(NCORES)), trace=trace, trace_cores=trace_cores
    )
    y = _assemble([r.results[c]["yt"] for c in range(NCORES)], inputs["bo"])
    return y, r


def kernel(**inputs):
    y, _ = _run(inputs, trace=False)
    return y


# revision 59
# speedup vs baseline: 1.1996x; 1.1996x over previous
"""Trainium2 Bass kernel for causal multi-head attention (B=4, T=2048, D=1024, H=16).

Sharding: 8 cores = 4 batches x 2 head-groups (8 heads each).
Per core pipeline (Tile framework, single SPMD program):
  phase 1(j): Q/K projections into transposed per-head-pair layout QT/KT [128=2*64, T],
           V projection into [t, 8*65] layout (65th col per head = ones, for rowsums)
  phase 2(j): per (q-range of 512, head-pair): causal flash attention in transposed
           layout: ST[k,q] = KT-slice^T @ QT-slice (row-packed pair of matmuls),
           PT = exp(ST) (ACT), causal triangle mask on diagonal 128-col strips (DVE),
           OT[hd+1, q] += [V|1]^T @ PT (bf16), normalize by approx-reciprocal rowsum.
  phase 3(j): output projection YT[dout, t] = Wo_gT^T @ OT, DMA'd straight from PSUM.
Phase 1(j+1) matmul chunks are emitted interleaved into phase 2(j) so the PE fills
its exp-wait gaps with projection work (phase 2 is ACT-bound; phases 1/3 PE-bound).
No collective: each core emits its partial YT [D, T]; the host adds the two partial
sums of each batch pair and adds the output bias.
"""

import numpy as np

B, T, D, H, HD = 4, 2048, 1024, 16, 64
NCORES = 8
NP = 4          # head pairs per core
NJ = 4          # q-ranges of 512
QW = 512
TB = T // 128   # 16

_CACHE = {}


def _build_nc():
    import concourse.mybir as mybir
    import concourse.tile as tile
    from concourse import bacc

    F32 = mybir.dt.float32
    BF16 = mybir.dt.bfloat16
    FP8 = mybir.dt.float8e4
    AF = mybir.ActivationFunctionType
    DR = mybir.MatmulPerfMode.DoubleRow

    nc = bacc.Bacc(None, target_bir_lowering=False)

    # Pin all activations to the one table holding Exp+Ln+Copy so the
    # act-table chooser can't thrash loads between the exp stream and the
    # exp(-ln(x)) reciprocal. Indices must match act_info.json, so other
    # tables are emptied rather than removed.
    import types as _types
    from concourse.hw_specs import get_activation_tables as _gat

    def _pinned_act_table_loads(self):
        import bass_rust as _bass_rust
        import concourse.mybir as _mybir
        has_activation = any(
            isinstance(i, _mybir.InstActivation)
            for b in self.main_func.blocks
            for i in b.instructions
        )
        if not has_activation:
            return
        tables = [
            (name, funcs if name == "natural_log_exp_and_others" else set())
            for name, funcs in _gat(self.m.arch).items()
        ]
        _bass_rust.insert_act_table_loads(self, tables)

    nc.insert_act_table_loads = _types.MethodType(_pinned_act_table_loads, nc)

    xt8_d = nc.declare_dram_parameter("xt8", [NJ, 128, 8 * QW], FP8, isOutput=False)
    xt_d = nc.declare_dram_parameter("xt", [NJ, 128, 8 * QW], BF16, isOutput=False)
    wq_d = nc.declare_dram_parameter("wq", [128, 8 * 512], FP8, isOutput=False)
    wk_d = nc.declare_dram_parameter("wk", [128, 8 * 512], FP8, isOutput=False)
    wv_d = nc.declare_dram_parameter("wv", [128, 8 * 512], BF16, isOutput=False)
    wo_d = nc.declare_dram_parameter("wo", [128, 4 * D], BF16, isOutput=False)
    mask_d = nc.declare_dram_parameter("mask", [128, 128], BF16, isOutput=False)
    yt_d = nc.declare_dram_parameter("yt", [D, T], F32, isOutput=True)

    with tile.TileContext(nc) as tc:
        with (
            tc.tile_pool(name="persist", bufs=1) as pers,
            tc.tile_pool(name="work", bufs=1) as work,
            tc.tile_pool(name="psum", bufs=1, space="PSUM") as psum,
        ):
            qt = pers.tile([128, NP, T], BF16)
            kt = pers.tile([128, NP, T], BF16)
            v = pers.tile([128, TB, 8 * 65], BF16)
            ot = pers.tile([128, NP, T], BF16)
            m0 = pers.tile([128, 128], BF16)
            wo = pers.tile([128, 4, D], BF16)
            wq = pers.tile([128, 8, 512], FP8)
            wk = pers.tile([128, 8, 512], FP8)
            wv = pers.tile([128, 8, 512], BF16)

            xs_tiles = {}
            xsb_tiles = {}

            def load_xs(j):
                t = work.tile([128, 8, QW], FP8, tag="xs", bufs=3)
                tb = work.tile([128, 8, QW], BF16, tag="xsb", bufs=3)
                nc.sync.dma_start(
                    out=t[:], in_=xt8_d[j].rearrange("p (c n) -> p c n", c=8)
                )
                nc.sync.dma_start(
                    out=tb[:], in_=xt_d[j].rearrange("p (c n) -> p c n", c=8)
                )
                xs_tiles[j] = t
                xsb_tiles[j] = tb

            # startup DMAs: weights are host-prearranged [128, ...] so each is
            # one fully contiguous transfer per partition
            nc.sync.dma_start(out=m0[:], in_=mask_d[:])
            nc.sync.dma_start(out=wq[:], in_=wq_d.rearrange("p (c n) -> p c n", c=8))
            nc.sync.dma_start(out=wk[:], in_=wk_d.rearrange("p (c n) -> p c n", c=8))
            load_xs(0)
            nc.sync.dma_start(out=wv[:], in_=wv_d.rearrange("p (c n) -> p c n", c=8))
            nc.sync.dma_start(out=wo[:], in_=wo_d.rearrange("p (c n) -> p c n", c=4))

            # Filler work is emitted as ~2-matmul micro-chunks: a whole 8-MM
            # chunk in the PE FIFO delays the next ST by up to ~1.8us, which
            # starves the ACT exp stream (the phase-2 critical path). Each
            # chunk is a pair of closures sharing one PSUM accumulator; the
            # pair stays adjacent in the drain list so at most two
            # accumulation groups are ever open on the "small" tag.
            # Weights are pre-scaled x128 on the host so they sit in e4m3's
            # normal range; the 1/128 is undone in the PSUM->SBUF copy.
            def qk_micros(j, p, w_sb, dst):
                st_ = {}

                def half(h):
                    if h == 0:
                        acc = psum.tile([128, QW], F32, tag="small", bufs=2)
                        st_["acc"] = acc
                    acc = st_["acc"]
                    for cp in (2 * h, 2 * h + 1):
                        nc.tensor.matmul(
                            acc[:],
                            w_sb[:, 2 * cp:2 * cp + 2, p * 128:(p + 1) * 128],
                            xs_tiles[j][:, 2 * cp:2 * cp + 2, :],
                            start=(cp == 0),
                            stop=(cp == 3),
                            perf_mode=DR,
                        )
                    if h == 1:
                        nc.vector.tensor_scalar_mul(
                            dst[:, p, j * QW:(j + 1) * QW], acc[:], 1.0 / 128.0
                        )
                return [lambda: half(0), lambda: half(1)]

            def v_micros(j, sub):
                # V path stays bf16: early tokens average few keys, so v
                # quantization error doesn't wash out like q/k error does
                i = 4 * j + sub
                st_ = {}

                def half(h):
                    if h == 0:
                        acc = psum.tile([128, QW], F32, tag="small", bufs=2)
                        st_["acc"] = acc
                    acc = st_["acc"]
                    for c in range(4 * h, 4 * h + 4):
                        nc.tensor.matmul(
                            acc[:],
                            xsb_tiles[j][:, c, sub * 128:(sub + 1) * 128],
                            wv[:, c, :],
                            start=(c == 0),
                            stop=(c == 7),
                        )
                    if h == 1:
                        vblk = v[:, i, :].rearrange("p (h c) -> p h c", c=65)
                        nc.vector.tensor_copy(
                            vblk[:, :, 0:64],
                            acc[:].rearrange("p (h c) -> p h c", c=64),
                        )
                        nc.gpsimd.memset(vblk[:, :, 64:65], 1.0)
                return [lambda: half(0), lambda: half(1)]

            def phase1_micros(j, v_early=False):
                ch = []
                for p in range(NP):
                    ch += qk_micros(j, p, wq, qt)
                    ch += qk_micros(j, p, wk, kt)
                    if v_early and p == 0:
                        for sub in range(4):
                            ch += v_micros(j, sub)
                if not v_early:
                    for sub in range(4):
                        ch += v_micros(j, sub)
                return ch

            # phase 1(0): only Q0/K0 up front — everything else interleaves
            # into phase 2(0)'s slots so the exp stream starts ~10us earlier
            for m_ in qk_micros(0, 0, wq, qt) + qk_micros(0, 0, wk, kt):
                m_()
            load_xs(1)
            # V blocks land just before their AVs; Q/K(p) just before pair p.
            # Targets are non-decreasing along the list (drain pops in order).
            leftover0 = (
                v_micros(0, 0)
                + qk_micros(0, 1, wq, qt) + v_micros(0, 1)
                + qk_micros(0, 1, wk, kt) + v_micros(0, 2)
                + v_micros(0, 3)
                + qk_micros(0, 2, wq, qt) + qk_micros(0, 2, wk, kt)
                + qk_micros(0, 3, wq, qt) + qk_micros(0, 3, wk, kt)
            )
            leftover0_targets = [0, 0, 1, 1, 1, 1, 2, 2, 2, 2,
                                 3, 3, 4, 4, 5, 5, 8, 8, 9, 9]

            def p3_micros(j, n):
                jrp = slice(j * QW, (j + 1) * QW)
                st_ = {}

                def half(h):
                    if h == 0:
                        yps = psum.tile([128, QW], F32, tag="small", bufs=2)
                        st_["y"] = yps
                    yps = st_["y"]
                    for c4 in (2 * h, 2 * h + 1):
                        nc.tensor.matmul(
                            yps[:],
                            wo[:, c4, n * 128:(n + 1) * 128],
                            ot[:, c4, jrp],
                            start=(c4 == 0), stop=(c4 == 3),
                        )
                    if h == 1:
                        ysb = work.tile([128, QW], F32, tag="ysb", bufs=3)
                        nc.vector.tensor_copy(ysb[:], yps[:])
                        nc.sync.dma_start(
                            out=yt_d[n * 128:(n + 1) * 128, jrp], in_=ysb[:]
                        )
                return [lambda: half(0), lambda: half(1)]

            def phase3_chunks(j):
                ch = []
                for n in range(8):
                    ch += p3_micros(j, n)
                return ch

            carry_kv = []
            for j in range(NJ):
                jr = slice(j * QW, (j + 1) * QW)
                # PE filler work for this j's ACT-bound attention stream:
                # previous j's output projection + next j's projections
                p3 = phase3_chunks(j - 1) if j > 0 else []
                if j + 2 < NJ:
                    load_xs(j + 2)
                nkb = 4 * j + 4
                slots = NP * nkb
                if j + 1 == NJ - 1:
                    # defer the last j's K/V projections into phase 2(3)
                    # itself: only its diagonal blocks (kb>=12) consume them,
                    # and j=3 is the ACT-bound stretch that starves the PE
                    p1 = []
                    for p in range(NP):
                        p1 += qk_micros(j + 1, p, wq, qt)
                    carry_kv = []
                    for p in range(NP):
                        carry_kv += qk_micros(j + 1, p, wk, kt)
                    for s in range(4):
                        carry_kv += v_micros(j + 1, s)
                elif j + 1 < NJ:
                    p1 = phase1_micros(j + 1)
                else:
                    p1 = []
                if j == NJ - 1:
                    # deferred K/V first (needed from kb=12 on), then the
                    # previous j's output projection spread across the rest
                    chunks = carry_kv + p3
                    targets = [1 + (i * 10) // len(carry_kv) for i in range(len(carry_kv))] + [
                        12 + (3 * i) for i in range(len(p3))
                    ]
                else:
                    # interleave micro-pairs of phase3(j-1) and phase1(j+1),
                    # keeping each chunk's two halves adjacent
                    chunks = []
                    for i in range(max(len(p3), len(p1)) // 2 + 1):
                        if 2 * i < len(p3):
                            chunks += p3[2 * i:2 * i + 2]
                        if 2 * i < len(p1):
                            chunks += p1[2 * i:2 * i + 2]
                    targets = [
                        (i + 1) * slots // (len(chunks) + 3)
                        for i in range(len(chunks))
                    ]
                    if j == 0:
                        # rest of phase 1(0), placed just ahead of its consumers
                        chunks = leftover0 + chunks
                        targets = leftover0_targets + [
                            max(t, 13) for t in targets
                        ]
                emitted = 0
                slot = 0
                ocps = []

                def emit_norm(p):
                    # 1/r = exp(-ln(r)): Ln/Exp share the pinned act table.
                    # Emitted one head-pair late so the ln never waits on the
                    # ocp copy inside the strict ACT FIFO.
                    lnr = work.tile([1, 1024], F32, tag="lnr", bufs=2)
                    nc.scalar.activation(lnr[:], ocps[p][64:65, :], AF.Ln)
                    rec = work.tile([1, 1024], F32, tag="rec", bufs=4)
                    nc.scalar.activation(rec[:], lnr[:], AF.Exp, scale=-1.0)
                    bc = work.tile([64, 1024], F32, tag="bc", bufs=3)
                    nc.gpsimd.partition_broadcast(bc[:, 0:QW], rec[:, 0:QW], channels=64)
                    nc.gpsimd.partition_broadcast(bc[:, QW:1024], rec[:, QW:1024], channels=64)
                    nc.vector.tensor_mul(ot[0:64, p, jr], ocps[p][0:64, 0:QW], bc[:, 0:QW])
                    nc.vector.tensor_mul(ot[64:128, p, jr], ocps[p][0:64, QW:1024], bc[:, QW:1024])

                # ---------------- phase 2(j) with phase 1(j+1) interleaved ----------
                for p in range(NP):
                    hA, hB = 2 * p, 2 * p + 1
                    o_A = psum.tile([65, QW], F32, tag="o", bufs=2)
                    o_B = psum.tile([65, QW], F32, tag="o", bufs=2)
                    for kb in range(nkb):
                        # drain filler BEFORE the kb body so producers land
                        # ahead of their phase-2 consumers
                        while emitted < len(chunks) and slot >= targets[emitted]:
                            chunks[emitted]()
                            emitted += 1
                        o = kb - 4 * j  # diagonal offset; < 0 means full block
                        lo = 128 * o if o > 0 else 0
                        st = psum.tile([128, 1024], F32, tag="st", bufs=2)
                        kcols = slice(kb * 128, (kb + 1) * 128)
                        qcols = slice(j * QW + lo, (j + 1) * QW)
                        nc.tensor.matmul(
                            st[:, lo:QW],
                            kt[0:64, p, kcols],
                            qt[0:64, p, qcols],
                            start=True, stop=True, tile_position=(0, 0),
                        )
                        nc.tensor.matmul(
                            st[:, QW + lo:2 * QW],
                            kt[64:128, p, kcols],
                            qt[64:128, p, qcols],
                            start=True, stop=True, tile_position=(64, 0),
                        )
                        pt = work.tile([128, 1024], BF16, tag="pt", bufs=3)
                        nc.scalar.activation(
                            pt[:].rearrange("p (h q) -> p h q", h=2)[:, :, lo:QW],
                            st[:].rearrange("p (h q) -> p h q", h=2)[:, :, lo:QW],
                            AF.Exp,
                        )
                        if o >= 0:
                            # only the leading 128-col strip of the valid range
                            # holds the causal triangle
                            nc.vector.tensor_mul(
                                pt[:, lo:lo + 128], pt[:, lo:lo + 128], m0[:]
                            )
                            nc.vector.tensor_mul(
                                pt[:, QW + lo:QW + lo + 128],
                                pt[:, QW + lo:QW + lo + 128],
                                m0[:],
                            )
                        nc.tensor.matmul(
                            o_A[:, lo:QW],
                            v[:, kb, hA * 65:(hA + 1) * 65],
                            pt[:, lo:QW],
                            start=(kb == 0), stop=(kb == nkb - 1),
                        )
                        nc.tensor.matmul(
                            o_B[:, lo:QW],
                            v[:, kb, hB * 65:(hB + 1) * 65],
                            pt[:, QW + lo:2 * QW],
                            start=(kb == 0), stop=(kb == nkb - 1),
                        )
                        slot += 1
                    # stage o out of PSUM promptly so the o slots free for the
                    # next head-pair (keeps PE from stalling / HAM warm)
                    ocp = work.tile([65, 1024], F32, tag="ocp", bufs=5)
                    nc.vector.tensor_copy(ocp[:, 0:QW], o_A[:])
                    nc.vector.tensor_copy(ocp[:, QW:1024], o_B[:])
                    ocps.append(ocp)
                    if p >= 1:
                        emit_norm(p - 1)
                emit_norm(NP - 1)
                while emitted < len(chunks):
                    chunks[emitted]()
                    emitted += 1

            # last j's output projection (the tail): pipeline two n-blocks so
            # the c=0..2 accumulations run while the final normalize drains
            jrp = slice((NJ - 1) * QW, NJ * QW)
            for npair in range(4):
                yy = []
                for n in (2 * npair, 2 * npair + 1):
                    yps = psum.tile([128, QW], F32, tag="small", bufs=2)
                    for c4 in range(3):
                        nc.tensor.matmul(
                            yps[:],
                            wo[:, c4, n * 128:(n + 1) * 128],
                            ot[:, c4, jrp],
                            start=(c4 == 0), stop=False,
                        )
                    yy.append(yps)
                for i, n in enumerate((2 * npair, 2 * npair + 1)):
                    yps = yy[i]
                    nc.tensor.matmul(
                        yps[:],
                        wo[:, 3, n * 128:(n + 1) * 128],
                        ot[:, 3, jrp],
                        start=False, stop=True,
                    )
                    ysb = work.tile([128, QW], F32, tag="ysb", bufs=3)
                    nc.vector.tensor_copy(ysb[:], yps[:])
                    nc.sync.dma_start(
                        out=yt_d[n * 128:(n + 1) * 128, jrp], in_=ysb[:]
                    )

    nc.finalize()
    return nc


def _prep_inputs(x, Wq, Wk, Wv, Wo, bo):
    """Build the 8 per-core input maps (host-side layout prep only)."""
    import ml_dtypes

    scale = 1.0 / np.sqrt(np.float32(HD))
    kr = np.arange(128, dtype=np.float32)[:, None]
    qc = np.arange(128, dtype=np.float32)[None, :]
    m0 = (qc >= kr).astype(ml_dtypes.bfloat16)

    FP8 = ml_dtypes.float8_e4m3  # TRN FP8_EXP4-compatible for |x| <= 240

    def xarr(xb, dtype):  # [T, D] -> [NJ, 128, 8*512], one contiguous DMA per j
        xt = xb.T  # [D, T]
        out = np.stack(
            [
                xt[:, j * QW:(j + 1) * QW]
                .reshape(8, 128, QW).transpose(1, 0, 2).reshape(128, 8 * QW)
                for j in range(NJ)
            ]
        )
        return np.ascontiguousarray(out).astype(dtype)

    xt8s = [xarr(np.clip(x[b], -240, 240), FP8) for b in range(B)]
    xts = [xarr(x[b], ml_dtypes.bfloat16) for b in range(B)]
    in_maps = []
    for c in range(NCORES):
        b, g = c // 2, c % 2
        hs = slice(g * 8, (g + 1) * 8)
        # x128 prescale keeps the small weights inside e4m3's normal range;
        # the kernel multiplies the projection PSUM by 1/128 when casting out.
        # layouts are [128, c*...] so each weight loads as one contiguous DMA
        def warr(wt, dtype):  # [D, 512] -> [128, 8*512], row p = concat_c w[c*128+p]
            return np.ascontiguousarray(
                wt.reshape(8, 128, 512).transpose(1, 0, 2).reshape(128, 8 * 512)
            ).astype(dtype)

        wqc = warr(Wq[hs].reshape(512, D).T * (scale * 128), FP8)
        wkc = warr(Wk[hs].reshape(512, D).T * 128, FP8)
        wvc = warr(Wv[hs].reshape(512, D).T, ml_dtypes.bfloat16)
        woc = np.ascontiguousarray(
            Wo[:, g * 512:(g + 1) * 512].T.reshape(4, 128, D).transpose(1, 0, 2).reshape(128, 4 * D)
        ).astype(ml_dtypes.bfloat16)
        in_maps.append(
            {"xt8": xt8s[b], "xt": xts[b], "wq": wqc, "wk": wkc, "wv": wvc,
             "wo": woc, "mask": m0}
        )
    return in_maps


def _assemble(yts, bo):
    """Sum the per-core partial outputs of each batch pair, add bias."""
    y = np.empty((B, T, D), np.float32)
    for b in range(B):
        y[b] = (yts[2 * b] + yts[2 * b + 1]).T
    y += bo.astype(np.float32)[None, None, :]
    return y


def _run(inputs, trace=False, trace_cores=None):
    from concourse.bass_utils import run_bass_kernel_spmd

    if "nc" not in _CACHE:
        _CACHE["nc"] = _build_nc()
    nc = _CACHE["nc"]
    in_maps = _prep_inputs(
        inputs["x"], inputs["Wq"], inputs["Wk"], inputs["Wv"], inputs["Wo"], inputs["bo"]
    )
    r = run_bass_kernel_spmd(
        nc, in_maps, list(range(NCORES)), trace=trace, trace_cores=trace_cores
    )
    y = _assemble([r.results[c]["yt"] for c in range(NCORES)], inputs["bo"])
    return y, r


def kernel(**inputs):
    y, _ = _run(inputs, trace=False)
    return y


# revision 60
# speedup vs baseline: 1.2339x; 1.0286x over previous
"""Trainium2 Bass kernel for causal multi-head attention (B=4, T=2048, D=1024, H=16).

Sharding: 8 cores = 4 batches x 2 head-groups (8 heads each).
Per core pipeline (Tile framework, single SPMD program):
  phase 1(j): Q/K projections into transposed per-head-pair layout QT/KT [128=2*64, T],
           V projection into [t, 8*65] layout (65th col per head = ones, for rowsums)
  phase 2(j): per (q-range of 512, head-pair): causal flash attention in transposed
           layout: ST[k,q] = KT-slice^T @ QT-slice (row-packed pair of matmuls),
           PT = exp(ST) (ACT), causal triangle mask on diagonal 128-col strips (DVE),
           OT[hd+1, q] += [V|1]^T @ PT (bf16), normalize by approx-reciprocal rowsum.
  phase 3(j): output projection YT[dout, t] = Wo_gT^T @ OT, DMA'd straight from PSUM.
Phase 1(j+1) matmul chunks are emitted interleaved into phase 2(j) so the PE fills
its exp-wait gaps with projection work (phase 2 is ACT-bound; phases 1/3 PE-bound).
No collective: each core emits its partial YT [D, T]; the host adds the two partial
sums of each batch pair and adds the output bias.
"""

import numpy as np

B, T, D, H, HD = 4, 2048, 1024, 16, 64
NCORES = 8
NP = 4          # head pairs per core
NJ = 4          # q-ranges of 512
QW = 512
TB = T // 128   # 16

_CACHE = {}


def _build_nc():
    import concourse.mybir as mybir
    import concourse.tile as tile
    from concourse import bacc

    F32 = mybir.dt.float32
    BF16 = mybir.dt.bfloat16
    FP8 = mybir.dt.float8e4
    AF = mybir.ActivationFunctionType
    DR = mybir.MatmulPerfMode.DoubleRow

    nc = bacc.Bacc(None, target_bir_lowering=False)

    # Pin all activations to the one table holding Exp+Ln+Copy so the
    # act-table chooser can't thrash loads between the exp stream and the
    # exp(-ln(x)) reciprocal. Indices must match act_info.json, so other
    # tables are emptied rather than removed.
    import types as _types
    from concourse.hw_specs import get_activation_tables as _gat

    def _pinned_act_table_loads(self):
        import bass_rust as _bass_rust
        import concourse.mybir as _mybir
        has_activation = any(
            isinstance(i, _mybir.InstActivation)
            for b in self.main_func.blocks
            for i in b.instructions
        )
        if not has_activation:
            return
        tables = [
            (name, funcs if name == "natural_log_exp_and_others" else set())
            for name, funcs in _gat(self.m.arch).items()
        ]
        _bass_rust.insert_act_table_loads(self, tables)

    nc.insert_act_table_loads = _types.MethodType(_pinned_act_table_loads, nc)

    xt8_d = nc.declare_dram_parameter("xt8", [NJ, 128, 8 * QW], FP8, isOutput=False)
    xt_d = nc.declare_dram_parameter("xt", [NJ, 128, 8 * QW], BF16, isOutput=False)
    wq_d = nc.declare_dram_parameter("wq", [128, 8 * 512], FP8, isOutput=False)
    wk_d = nc.declare_dram_parameter("wk", [128, 8 * 512], FP8, isOutput=False)
    wv_d = nc.declare_dram_parameter("wv", [128, 8 * 512], BF16, isOutput=False)
    wo_d = nc.declare_dram_parameter("wo", [128, 4 * D], BF16, isOutput=False)
    mask_d = nc.declare_dram_parameter("mask", [128, 256], BF16, isOutput=False)
    yt_d = nc.declare_dram_parameter("yt", [D, T], F32, isOutput=True)

    with tile.TileContext(nc) as tc:
        with (
            tc.tile_pool(name="persist", bufs=1) as pers,
            tc.tile_pool(name="work", bufs=1) as work,
            tc.tile_pool(name="psum", bufs=1, space="PSUM") as psum,
        ):
            qt = pers.tile([128, NP, T], BF16)
            kt = pers.tile([128, NP, T], BF16)
            v = pers.tile([128, TB, 8 * 65], BF16)
            ot = pers.tile([128, NP, T], BF16)
            m0 = pers.tile([128, 256], BF16)
            wo = pers.tile([128, 4, D], BF16)
            wq = pers.tile([128, 8, 512], FP8)
            wk = pers.tile([128, 8, 512], FP8)
            wv = pers.tile([128, 8, 512], BF16)

            xs_tiles = {}
            xsb_tiles = {}

            def load_xs(j):
                t = work.tile([128, 8, QW], FP8, tag="xs", bufs=3)
                tb = work.tile([128, 8, QW], BF16, tag="xsb", bufs=3)
                nc.sync.dma_start(
                    out=t[:], in_=xt8_d[j].rearrange("p (c n) -> p c n", c=8)
                )
                nc.sync.dma_start(
                    out=tb[:], in_=xt_d[j].rearrange("p (c n) -> p c n", c=8)
                )
                xs_tiles[j] = t
                xsb_tiles[j] = tb

            # startup DMAs: weights are host-prearranged [128, ...] so each is
            # one fully contiguous transfer per partition
            nc.sync.dma_start(out=m0[:], in_=mask_d[:])
            nc.sync.dma_start(out=wq[:], in_=wq_d.rearrange("p (c n) -> p c n", c=8))
            nc.sync.dma_start(out=wk[:], in_=wk_d.rearrange("p (c n) -> p c n", c=8))
            load_xs(0)
            nc.sync.dma_start(out=wv[:], in_=wv_d.rearrange("p (c n) -> p c n", c=8))
            nc.sync.dma_start(out=wo[:], in_=wo_d.rearrange("p (c n) -> p c n", c=4))

            # Filler work is emitted as ~2-matmul micro-chunks: a whole 8-MM
            # chunk in the PE FIFO delays the next ST by up to ~1.8us, which
            # starves the ACT exp stream (the phase-2 critical path). Each
            # chunk is a pair of closures sharing one PSUM accumulator; the
            # pair stays adjacent in the drain list so at most two
            # accumulation groups are ever open on the "small" tag.
            # Weights are pre-scaled x128 on the host so they sit in e4m3's
            # normal range; the 1/128 is undone in the PSUM->SBUF copy.
            def qk_micros(j, p, w_sb, dst):
                st_ = {}

                def half(h):
                    if h == 0:
                        acc = psum.tile([128, QW], F32, tag="small", bufs=2)
                        st_["acc"] = acc
                    acc = st_["acc"]
                    for cp in (2 * h, 2 * h + 1):
                        nc.tensor.matmul(
                            acc[:],
                            w_sb[:, 2 * cp:2 * cp + 2, p * 128:(p + 1) * 128],
                            xs_tiles[j][:, 2 * cp:2 * cp + 2, :],
                            start=(cp == 0),
                            stop=(cp == 3),
                            perf_mode=DR,
                        )
                    if h == 1:
                        nc.vector.tensor_scalar_mul(
                            dst[:, p, j * QW:(j + 1) * QW], acc[:], 1.0 / 128.0
                        )
                return [lambda: half(0), lambda: half(1)]

            def v_micros(j, sub):
                # V path stays bf16: early tokens average few keys, so v
                # quantization error doesn't wash out like q/k error does
                i = 4 * j + sub
                st_ = {}

                def half(h):
                    if h == 0:
                        acc = psum.tile([128, QW], F32, tag="small", bufs=2)
                        st_["acc"] = acc
                    acc = st_["acc"]
                    for c in range(4 * h, 4 * h + 4):
                        nc.tensor.matmul(
                            acc[:],
                            xsb_tiles[j][:, c, sub * 128:(sub + 1) * 128],
                            wv[:, c, :],
                            start=(c == 0),
                            stop=(c == 7),
                        )
                    if h == 1:
                        vblk = v[:, i, :].rearrange("p (h c) -> p h c", c=65)
                        nc.vector.tensor_copy(
                            vblk[:, :, 0:64],
                            acc[:].rearrange("p (h c) -> p h c", c=64),
                        )
                        nc.gpsimd.memset(vblk[:, :, 64:65], 1.0)
                return [lambda: half(0), lambda: half(1)]

            def phase1_micros(j, v_early=False):
                ch = []
                for p in range(NP):
                    ch += qk_micros(j, p, wq, qt)
                    ch += qk_micros(j, p, wk, kt)
                    if v_early and p == 0:
                        for sub in range(4):
                            ch += v_micros(j, sub)
                if not v_early:
                    for sub in range(4):
                        ch += v_micros(j, sub)
                return ch

            # phase 1(0): only Q0/K0 up front — everything else interleaves
            # into phase 2(0)'s slots so the exp stream starts ~10us earlier
            for m_ in qk_micros(0, 0, wq, qt) + qk_micros(0, 0, wk, kt):
                m_()
            load_xs(1)
            # V blocks land just before their AVs; Q/K(p) just before pair p.
            # Targets are non-decreasing along the list (drain pops in order).
            leftover0 = (
                v_micros(0, 0)
                + qk_micros(0, 1, wq, qt) + v_micros(0, 1)
                + qk_micros(0, 1, wk, kt) + v_micros(0, 2)
                + v_micros(0, 3)
                + qk_micros(0, 2, wq, qt) + qk_micros(0, 2, wk, kt)
                + qk_micros(0, 3, wq, qt) + qk_micros(0, 3, wk, kt)
            )
            leftover0_targets = [0, 0, 1, 1, 1, 1, 2, 2, 2, 2,
                                 3, 3, 4, 4, 5, 5, 8, 8, 9, 9]

            def p3_micros(j, n):
                jrp = slice(j * QW, (j + 1) * QW)
                st_ = {}

                def half(h):
                    if h == 0:
                        yps = psum.tile([128, QW], F32, tag="small", bufs=2)
                        st_["y"] = yps
                    yps = st_["y"]
                    for c4 in (2 * h, 2 * h + 1):
                        nc.tensor.matmul(
                            yps[:],
                            wo[:, c4, n * 128:(n + 1) * 128],
                            ot[:, c4, jrp],
                            start=(c4 == 0), stop=(c4 == 3),
                        )
                    if h == 1:
                        ysb = work.tile([128, QW], F32, tag="ysb", bufs=3)
                        nc.vector.tensor_copy(ysb[:], yps[:])
                        nc.sync.dma_start(
                            out=yt_d[n * 128:(n + 1) * 128, jrp], in_=ysb[:]
                        )
                return [lambda: half(0), lambda: half(1)]

            def phase3_chunks(j):
                ch = []
                for n in range(8):
                    ch += p3_micros(j, n)
                return ch

            carry_kv = []
            for j in range(NJ):
                jr = slice(j * QW, (j + 1) * QW)
                # PE filler work for this j's ACT-bound attention stream:
                # previous j's output projection + next j's projections
                p3 = phase3_chunks(j - 1) if j > 0 else []
                if j + 2 < NJ:
                    load_xs(j + 2)
                nkb = 4 * j + 4
                slots = NP * nkb
                if j + 1 == NJ - 1:
                    # defer the last j's K/V projections into phase 2(3)
                    # itself: only its diagonal blocks (kb>=12) consume them,
                    # and j=3 is the ACT-bound stretch that starves the PE
                    p1 = []
                    for p in range(NP):
                        p1 += qk_micros(j + 1, p, wq, qt)
                    carry_kv = []
                    for p in range(NP):
                        carry_kv += qk_micros(j + 1, p, wk, kt)
                    for s in range(4):
                        carry_kv += v_micros(j + 1, s)
                elif j + 1 < NJ:
                    p1 = phase1_micros(j + 1)
                else:
                    p1 = []
                if j == NJ - 1:
                    # deferred K/V first (needed from kb=12 on), then the
                    # previous j's output projection spread across the rest
                    chunks = carry_kv + p3
                    targets = [1 + (i * 10) // len(carry_kv) for i in range(len(carry_kv))] + [
                        12 + (3 * i) for i in range(len(p3))
                    ]
                else:
                    # interleave micro-pairs of phase3(j-1) and phase1(j+1),
                    # keeping each chunk's two halves adjacent
                    chunks = []
                    for i in range(max(len(p3), len(p1)) // 2 + 1):
                        if 2 * i < len(p3):
                            chunks += p3[2 * i:2 * i + 2]
                        if 2 * i < len(p1):
                            chunks += p1[2 * i:2 * i + 2]
                    targets = [
                        (i + 1) * slots // (len(chunks) + 3)
                        for i in range(len(chunks))
                    ]
                    if j == 0:
                        # rest of phase 1(0), placed just ahead of its consumers
                        chunks = leftover0 + chunks
                        targets = leftover0_targets + [
                            max(t, 13) for t in targets
                        ]
                emitted = 0
                slot = 0
                ocps = []

                def emit_norm(p):
                    # 1/r = exp(-ln(r)): Ln/Exp share the pinned act table.
                    # Emitted one head-pair late so the ln never waits on the
                    # ocp copy inside the strict ACT FIFO.
                    lnr = work.tile([1, 1024], F32, tag="lnr", bufs=2)
                    nc.scalar.activation(lnr[:], ocps[p][64:65, :], AF.Ln)
                    rec = work.tile([1, 1024], F32, tag="rec", bufs=4)
                    nc.scalar.activation(rec[:], lnr[:], AF.Exp, scale=-1.0)
                    bc = work.tile([64, 1024], F32, tag="bc", bufs=3)
                    nc.gpsimd.partition_broadcast(bc[:, 0:QW], rec[:, 0:QW], channels=64)
                    nc.gpsimd.partition_broadcast(bc[:, QW:1024], rec[:, QW:1024], channels=64)
                    nc.vector.tensor_mul(ot[0:64, p, jr], ocps[p][0:64, 0:QW], bc[:, 0:QW])
                    nc.vector.tensor_mul(ot[64:128, p, jr], ocps[p][0:64, QW:1024], bc[:, QW:1024])

                # ---------------- phase 2(j) with phase 1(j+1) interleaved ----------
                for p in range(NP):
                    hA, hB = 2 * p, 2 * p + 1
                    o_A = psum.tile([65, QW], F32, tag="o", bufs=2)
                    o_B = psum.tile([65, QW], F32, tag="o", bufs=2)
                    for kb in range(nkb):
                        # drain filler BEFORE the kb body so producers land
                        # ahead of their phase-2 consumers
                        while emitted < len(chunks) and slot >= targets[emitted]:
                            chunks[emitted]()
                            emitted += 1
                        o = kb - 4 * j  # diagonal offset; < 0 means full block
                        lo = 128 * o if o > 0 else 0
                        st = psum.tile([128, 1024], F32, tag="st", bufs=2)
                        kcols = slice(kb * 128, (kb + 1) * 128)
                        qcols = slice(j * QW + lo, (j + 1) * QW)
                        nc.tensor.matmul(
                            st[:, lo:QW],
                            kt[0:64, p, kcols],
                            qt[0:64, p, qcols],
                            start=True, stop=True, tile_position=(0, 0),
                        )
                        nc.tensor.matmul(
                            st[:, QW + lo:2 * QW],
                            kt[64:128, p, kcols],
                            qt[64:128, p, qcols],
                            start=True, stop=True, tile_position=(64, 0),
                        )
                        pt = work.tile([128, 1024], BF16, tag="pt", bufs=3)
                        nc.scalar.activation(
                            pt[:].rearrange("p (h q) -> p h q", h=2)[:, :, lo:QW],
                            st[:].rearrange("p (h q) -> p h q", h=2)[:, :, lo:QW],
                            AF.Exp,
                        )
                        if o >= 0:
                            # only the leading 128-col strip of the valid range
                            # holds the causal triangle; one strided op covers
                            # both heads' strips (m0 is the triangle, tiled x2)
                            strips = pt[:].rearrange(
                                "p (h q) -> p h q", h=2
                            )[:, :, lo:lo + 128]
                            nc.vector.tensor_mul(
                                strips,
                                strips,
                                m0[:].rearrange("p (h q) -> p h q", h=2),
                            )
                        nc.tensor.matmul(
                            o_A[:, lo:QW],
                            v[:, kb, hA * 65:(hA + 1) * 65],
                            pt[:, lo:QW],
                            start=(kb == 0), stop=(kb == nkb - 1),
                        )
                        nc.tensor.matmul(
                            o_B[:, lo:QW],
                            v[:, kb, hB * 65:(hB + 1) * 65],
                            pt[:, QW + lo:2 * QW],
                            start=(kb == 0), stop=(kb == nkb - 1),
                        )
                        slot += 1
                    # stage o out of PSUM promptly so the o slots free for the
                    # next head-pair (keeps PE from stalling / HAM warm)
                    ocp = work.tile([65, 1024], F32, tag="ocp", bufs=5)
                    nc.vector.tensor_copy(ocp[:, 0:QW], o_A[:])
                    nc.vector.tensor_copy(ocp[:, QW:1024], o_B[:])
                    ocps.append(ocp)
                    if p >= 1:
                        emit_norm(p - 1)
                emit_norm(NP - 1)
                while emitted < len(chunks):
                    chunks[emitted]()
                    emitted += 1

            # last j's output projection (the tail): pipeline two n-blocks so
            # the c=0..2 accumulations run while the final normalize drains
            jrp = slice((NJ - 1) * QW, NJ * QW)
            for npair in range(4):
                yy = []
                for n in (2 * npair, 2 * npair + 1):
                    yps = psum.tile([128, QW], F32, tag="small", bufs=2)
                    for c4 in range(3):
                        nc.tensor.matmul(
                            yps[:],
                            wo[:, c4, n * 128:(n + 1) * 128],
                            ot[:, c4, jrp],
                            start=(c4 == 0), stop=False,
                        )
                    yy.append(yps)
                for i, n in enumerate((2 * npair, 2 * npair + 1)):
                    yps = yy[i]
                    nc.tensor.matmul(
                        yps[:],
                        wo[:, 3, n * 128:(n + 1) * 128],
                        ot[:, 3, jrp],
                        start=False, stop=True,
                    )
                    ysb = work.tile([128, QW], F32, tag="ysb", bufs=3)
                    nc.vector.tensor_copy(ysb[:], yps[:])
                    nc.sync.dma_start(
                        out=yt_d[n * 128:(n + 1) * 128, jrp], in_=ysb[:]
                    )

    nc.finalize()
    return nc


def _prep_inputs(x, Wq, Wk, Wv, Wo, bo):
    """Build the 8 per-core input maps (host-side layout prep only)."""
    import ml_dtypes

    scale = 1.0 / np.sqrt(np.float32(HD))
    kr = np.arange(128, dtype=np.float32)[:, None]
    qc = np.arange(128, dtype=np.float32)[None, :]
    tri = (qc >= kr)
    m0 = np.concatenate([tri, tri], axis=1).astype(ml_dtypes.bfloat16)

    FP8 = ml_dtypes.float8_e4m3  # TRN FP8_EXP4-compatible for |x| <= 240

    def xarr(xb, dtype):  # [T, D] -> [NJ, 128, 8*512], one contiguous DMA per j
        xt = xb.T  # [D, T]
        out = np.stack(
            [
                xt[:, j * QW:(j + 1) * QW]
                .reshape(8, 128, QW).transpose(1, 0, 2).reshape(128, 8 * QW)
                for j in range(NJ)
            ]
        )
        return np.ascontiguousarray(out).astype(dtype)

    xt8s = [xarr(np.clip(x[b], -240, 240), FP8) for b in range(B)]
    xts = [xarr(x[b], ml_dtypes.bfloat16) for b in range(B)]
    in_maps = []
    for c in range(NCORES):
        b, g = c // 2, c % 2
        hs = slice(g * 8, (g + 1) * 8)
        # x128 prescale keeps the small weights inside e4m3's normal range;
        # the kernel multiplies the projection PSUM by 1/128 when casting out.
        # layouts are [128, c*...] so each weight loads as one contiguous DMA
        def warr(wt, dtype):  # [D, 512] -> [128, 8*512], row p = concat_c w[c*128+p]
            return np.ascontiguousarray(
                wt.reshape(8, 128, 512).transpose(1, 0, 2).reshape(128, 8 * 512)
            ).astype(dtype)

        wqc = warr(Wq[hs].reshape(512, D).T * (scale * 128), FP8)
        wkc = warr(Wk[hs].reshape(512, D).T * 128, FP8)
        wvc = warr(Wv[hs].reshape(512, D).T, ml_dtypes.bfloat16)
        woc = np.ascontiguousarray(
            Wo[:, g * 512:(g + 1) * 512].T.reshape(4, 128, D).transpose(1, 0, 2).reshape(128, 4 * D)
        ).astype(ml_dtypes.bfloat16)
        in_maps.append(
            {"xt8": xt8s[b], "xt": xts[b], "wq": wqc, "wk": wkc, "wv": wvc,
             "wo": woc, "mask": m0}
        )
    return in_maps


def _assemble(yts, bo):
    """Sum the per-core partial outputs of each batch pair, add bias."""
    y = np.empty((B, T, D), np.float32)
    for b in range(B):
        y[b] = (yts[2 * b] + yts[2 * b + 1]).T
    y += bo.astype(np.float32)[None, None, :]
    return y


def _run(inputs, trace=False, trace_cores=None):
    from concourse.bass_utils import run_bass_kernel_spmd

    if "nc" not in _CACHE:
        _CACHE["nc"] = _build_nc()
    nc = _CACHE["nc"]
    in_maps = _prep_inputs(
        inputs["x"], inputs["Wq"], inputs["Wk"], inputs["Wv"], inputs["Wo"], inputs["bo"]
    )
    r = run_bass_kernel_spmd(
        nc, in_maps, list(range(NCORES)), trace=trace, trace_cores=trace_cores
    )
    y = _assemble([r.results[c]["yt"] for c in range(NCORES)], inputs["bo"])
    return y, r


def kernel(**inputs):
    y, _ = _run(inputs, trace=False)
    return y


# revision 61
# speedup vs baseline: 1.2429x; 1.0073x over previous
"""Trainium2 Bass kernel for causal multi-head attention (B=4, T=2048, D=1024, H=16).

Sharding: 8 cores = 4 batches x 2 head-groups (8 heads each).
Per core pipeline (Tile framework, single SPMD program):
  phase 1(j): Q/K projections into transposed per-head-pair layout QT/KT [128=2*64, T],
           V projection into [t, 8*65] layout (65th col per head = ones, for rowsums)
  phase 2(j): per (q-range of 512, head-pair): causal flash attention in transposed
           layout: ST[k,q] = KT-slice^T @ QT-slice (row-packed pair of matmuls),
           PT = exp(ST) (ACT), causal triangle mask on diagonal 128-col strips (DVE),
           OT[hd+1, q] += [V|1]^T @ PT (bf16), normalize by approx-reciprocal rowsum.
  phase 3(j): output projection YT[dout, t] = Wo_gT^T @ OT, DMA'd straight from PSUM.
Phase 1(j+1) matmul chunks are emitted interleaved into phase 2(j) so the PE fills
its exp-wait gaps with projection work (phase 2 is ACT-bound; phases 1/3 PE-bound).
No collective: each core emits its partial YT [D, T]; the host adds the two partial
sums of each batch pair and adds the output bias.
"""

import numpy as np

B, T, D, H, HD = 4, 2048, 1024, 16, 64
NCORES = 8
NP = 4          # head pairs per core
NJ = 4          # q-ranges of 512
QW = 512
TB = T // 128   # 16

_CACHE = {}


def _build_nc():
    import concourse.mybir as mybir
    import concourse.tile as tile
    from concourse import bacc

    F32 = mybir.dt.float32
    BF16 = mybir.dt.bfloat16
    FP8 = mybir.dt.float8e4
    AF = mybir.ActivationFunctionType
    DR = mybir.MatmulPerfMode.DoubleRow

    nc = bacc.Bacc(None, target_bir_lowering=False)

    # Pin all activations to the one table holding Exp+Ln+Copy so the
    # act-table chooser can't thrash loads between the exp stream and the
    # exp(-ln(x)) reciprocal. Indices must match act_info.json, so other
    # tables are emptied rather than removed.
    import types as _types
    from concourse.hw_specs import get_activation_tables as _gat

    def _pinned_act_table_loads(self):
        import bass_rust as _bass_rust
        import concourse.mybir as _mybir
        has_activation = any(
            isinstance(i, _mybir.InstActivation)
            for b in self.main_func.blocks
            for i in b.instructions
        )
        if not has_activation:
            return
        tables = [
            (name, funcs if name == "natural_log_exp_and_others" else set())
            for name, funcs in _gat(self.m.arch).items()
        ]
        _bass_rust.insert_act_table_loads(self, tables)

    nc.insert_act_table_loads = _types.MethodType(_pinned_act_table_loads, nc)

    xt8_d = nc.declare_dram_parameter("xt8", [NJ, 128, 8 * QW], FP8, isOutput=False)
    xt_d = nc.declare_dram_parameter("xt", [NJ, 128, 8 * QW], BF16, isOutput=False)
    wq_d = nc.declare_dram_parameter("wq", [128, 8 * 512], FP8, isOutput=False)
    wk_d = nc.declare_dram_parameter("wk", [128, 8 * 512], FP8, isOutput=False)
    wv_d = nc.declare_dram_parameter("wv", [128, 8 * 512], BF16, isOutput=False)
    wo_d = nc.declare_dram_parameter("wo", [128, 4 * D], BF16, isOutput=False)
    mask_d = nc.declare_dram_parameter("mask", [128, 256], BF16, isOutput=False)
    yt_d = nc.declare_dram_parameter("yt", [D, T], F32, isOutput=True)

    with tile.TileContext(nc) as tc:
        with (
            tc.tile_pool(name="persist", bufs=1) as pers,
            tc.tile_pool(name="work", bufs=1) as work,
            tc.tile_pool(name="psum", bufs=1, space="PSUM") as psum,
        ):
            qt = pers.tile([128, NP, T], BF16)
            kt = pers.tile([128, NP, T], BF16)
            v = pers.tile([128, TB, 8 * 65], BF16)
            ot = pers.tile([128, NP, T], BF16)
            m0 = pers.tile([128, 256], BF16)
            wo = pers.tile([128, 4, D], BF16)
            wq = pers.tile([128, 8, 512], FP8)
            wk = pers.tile([128, 8, 512], FP8)
            wv = pers.tile([128, 8, 512], BF16)

            # all per-head ones-columns of V in one strided memset (the V
            # copies never touch column 64, so this never gets overwritten)
            nc.gpsimd.memset(
                v[:].rearrange("p i (h c) -> p i h c", c=65)[:, :, :, 64:65], 1.0
            )

            xs_tiles = {}
            xsb_tiles = {}

            def load_xs(j):
                t = work.tile([128, 8, QW], FP8, tag="xs", bufs=3)
                tb = work.tile([128, 8, QW], BF16, tag="xsb", bufs=3)
                nc.sync.dma_start(
                    out=t[:], in_=xt8_d[j].rearrange("p (c n) -> p c n", c=8)
                )
                nc.sync.dma_start(
                    out=tb[:], in_=xt_d[j].rearrange("p (c n) -> p c n", c=8)
                )
                xs_tiles[j] = t
                xsb_tiles[j] = tb

            # startup DMAs: weights are host-prearranged [128, ...] so each is
            # one fully contiguous transfer per partition
            nc.sync.dma_start(out=m0[:], in_=mask_d[:])
            nc.sync.dma_start(out=wq[:], in_=wq_d.rearrange("p (c n) -> p c n", c=8))
            nc.sync.dma_start(out=wk[:], in_=wk_d.rearrange("p (c n) -> p c n", c=8))
            load_xs(0)
            nc.sync.dma_start(out=wv[:], in_=wv_d.rearrange("p (c n) -> p c n", c=8))
            nc.sync.dma_start(out=wo[:], in_=wo_d.rearrange("p (c n) -> p c n", c=4))

            # Filler work is emitted as ~2-matmul micro-chunks: a whole 8-MM
            # chunk in the PE FIFO delays the next ST by up to ~1.8us, which
            # starves the ACT exp stream (the phase-2 critical path). Each
            # chunk is a pair of closures sharing one PSUM accumulator; the
            # pair stays adjacent in the drain list so at most two
            # accumulation groups are ever open on the "small" tag.
            # Weights are pre-scaled x128 on the host so they sit in e4m3's
            # normal range; the 1/128 is undone in the PSUM->SBUF copy.
            def qk_micros(j, p, w_sb, dst):
                st_ = {}

                def half(h):
                    if h == 0:
                        acc = psum.tile([128, QW], F32, tag="small", bufs=2)
                        st_["acc"] = acc
                    acc = st_["acc"]
                    for cp in (2 * h, 2 * h + 1):
                        nc.tensor.matmul(
                            acc[:],
                            w_sb[:, 2 * cp:2 * cp + 2, p * 128:(p + 1) * 128],
                            xs_tiles[j][:, 2 * cp:2 * cp + 2, :],
                            start=(cp == 0),
                            stop=(cp == 3),
                            perf_mode=DR,
                        )
                    if h == 1:
                        nc.vector.tensor_scalar_mul(
                            dst[:, p, j * QW:(j + 1) * QW], acc[:], 1.0 / 128.0
                        )
                return [lambda: half(0), lambda: half(1)]

            def v_micros(j, sub):
                # V path stays bf16: early tokens average few keys, so v
                # quantization error doesn't wash out like q/k error does
                i = 4 * j + sub
                st_ = {}

                def half(h):
                    if h == 0:
                        acc = psum.tile([128, QW], F32, tag="small", bufs=2)
                        st_["acc"] = acc
                    acc = st_["acc"]
                    for c in range(4 * h, 4 * h + 4):
                        nc.tensor.matmul(
                            acc[:],
                            xsb_tiles[j][:, c, sub * 128:(sub + 1) * 128],
                            wv[:, c, :],
                            start=(c == 0),
                            stop=(c == 7),
                        )
                    if h == 1:
                        vblk = v[:, i, :].rearrange("p (h c) -> p h c", c=65)
                        nc.vector.tensor_copy(
                            vblk[:, :, 0:64],
                            acc[:].rearrange("p (h c) -> p h c", c=64),
                        )
                return [lambda: half(0), lambda: half(1)]

            def phase1_micros(j, v_early=False):
                ch = []
                for p in range(NP):
                    ch += qk_micros(j, p, wq, qt)
                    ch += qk_micros(j, p, wk, kt)
                    if v_early and p == 0:
                        for sub in range(4):
                            ch += v_micros(j, sub)
                if not v_early:
                    for sub in range(4):
                        ch += v_micros(j, sub)
                return ch

            # phase 1(0): only Q0/K0 up front — everything else interleaves
            # into phase 2(0)'s slots so the exp stream starts ~10us earlier
            for m_ in qk_micros(0, 0, wq, qt) + qk_micros(0, 0, wk, kt):
                m_()
            load_xs(1)
            # V blocks land just before their AVs; Q/K(p) just before pair p.
            # Targets are non-decreasing along the list (drain pops in order).
            leftover0 = (
                v_micros(0, 0)
                + qk_micros(0, 1, wq, qt) + v_micros(0, 1)
                + qk_micros(0, 1, wk, kt) + v_micros(0, 2)
                + v_micros(0, 3)
                + qk_micros(0, 2, wq, qt) + qk_micros(0, 2, wk, kt)
                + qk_micros(0, 3, wq, qt) + qk_micros(0, 3, wk, kt)
            )
            leftover0_targets = [0, 0, 1, 1, 1, 1, 2, 2, 2, 2,
                                 3, 3, 4, 4, 5, 5, 8, 8, 9, 9]

            def p3_micros(j, n):
                jrp = slice(j * QW, (j + 1) * QW)
                st_ = {}

                def half(h):
                    if h == 0:
                        yps = psum.tile([128, QW], F32, tag="small", bufs=2)
                        st_["y"] = yps
                    yps = st_["y"]
                    for c4 in (2 * h, 2 * h + 1):
                        nc.tensor.matmul(
                            yps[:],
                            wo[:, c4, n * 128:(n + 1) * 128],
                            ot[:, c4, jrp],
                            start=(c4 == 0), stop=(c4 == 3),
                        )
                    if h == 1:
                        ysb = work.tile([128, QW], F32, tag="ysb", bufs=3)
                        nc.vector.tensor_copy(ysb[:], yps[:])
                        nc.sync.dma_start(
                            out=yt_d[n * 128:(n + 1) * 128, jrp], in_=ysb[:]
                        )
                return [lambda: half(0), lambda: half(1)]

            def phase3_chunks(j):
                ch = []
                for n in range(8):
                    ch += p3_micros(j, n)
                return ch

            carry_kv = []
            for j in range(NJ):
                jr = slice(j * QW, (j + 1) * QW)
                # PE filler work for this j's ACT-bound attention stream:
                # previous j's output projection + next j's projections
                p3 = phase3_chunks(j - 1) if j > 0 else []
                if j + 2 < NJ:
                    load_xs(j + 2)
                nkb = 4 * j + 4
                slots = NP * nkb
                if j + 1 == NJ - 1:
                    # defer the last j's K/V projections into phase 2(3)
                    # itself: only its diagonal blocks (kb>=12) consume them,
                    # and j=3 is the ACT-bound stretch that starves the PE
                    p1 = []
                    for p in range(NP):
                        p1 += qk_micros(j + 1, p, wq, qt)
                    carry_kv = []
                    for p in range(NP):
                        carry_kv += qk_micros(j + 1, p, wk, kt)
                    for s in range(4):
                        carry_kv += v_micros(j + 1, s)
                elif j + 1 < NJ:
                    p1 = phase1_micros(j + 1)
                else:
                    p1 = []
                if j == NJ - 1:
                    # deferred K/V first (needed from kb=12 on), then the
                    # previous j's output projection spread across the rest
                    chunks = carry_kv + p3
                    targets = [1 + (i * 10) // len(carry_kv) for i in range(len(carry_kv))] + [
                        12 + (3 * i) for i in range(len(p3))
                    ]
                else:
                    # interleave micro-pairs of phase3(j-1) and phase1(j+1),
                    # keeping each chunk's two halves adjacent
                    chunks = []
                    for i in range(max(len(p3), len(p1)) // 2 + 1):
                        if 2 * i < len(p3):
                            chunks += p3[2 * i:2 * i + 2]
                        if 2 * i < len(p1):
                            chunks += p1[2 * i:2 * i + 2]
                    targets = [
                        (i + 1) * slots // (len(chunks) + 3)
                        for i in range(len(chunks))
                    ]
                    if j == 0:
                        # rest of phase 1(0), placed just ahead of its consumers
                        chunks = leftover0 + chunks
                        targets = leftover0_targets + [
                            max(t, 13) for t in targets
                        ]
                emitted = 0
                slot = 0
                ocps = []

                def emit_norm(p):
                    # 1/r = exp(-ln(r)): Ln/Exp share the pinned act table.
                    # Emitted one head-pair late so the ln never waits on the
                    # ocp copy inside the strict ACT FIFO.
                    lnr = work.tile([1, 1024], F32, tag="lnr", bufs=2)
                    nc.scalar.activation(lnr[:], ocps[p][64:65, :], AF.Ln)
                    rec = work.tile([1, 1024], F32, tag="rec", bufs=4)
                    nc.scalar.activation(rec[:], lnr[:], AF.Exp, scale=-1.0)
                    bc = work.tile([64, 1024], F32, tag="bc", bufs=3)
                    nc.gpsimd.partition_broadcast(bc[:], rec[:], channels=64)
                    nc.vector.tensor_mul(ot[0:64, p, jr], ocps[p][0:64, 0:QW], bc[:, 0:QW])
                    nc.vector.tensor_mul(ot[64:128, p, jr], ocps[p][0:64, QW:1024], bc[:, QW:1024])

                # ---------------- phase 2(j) with phase 1(j+1) interleaved ----------
                for p in range(NP):
                    hA, hB = 2 * p, 2 * p + 1
                    o_A = psum.tile([65, QW], F32, tag="o", bufs=2)
                    o_B = psum.tile([65, QW], F32, tag="o", bufs=2)
                    for kb in range(nkb):
                        # drain filler BEFORE the kb body so producers land
                        # ahead of their phase-2 consumers
                        while emitted < len(chunks) and slot >= targets[emitted]:
                            chunks[emitted]()
                            emitted += 1
                        o = kb - 4 * j  # diagonal offset; < 0 means full block
                        lo = 128 * o if o > 0 else 0
                        st = psum.tile([128, 1024], F32, tag="st", bufs=2)
                        kcols = slice(kb * 128, (kb + 1) * 128)
                        qcols = slice(j * QW + lo, (j + 1) * QW)
                        nc.tensor.matmul(
                            st[:, lo:QW],
                            kt[0:64, p, kcols],
                            qt[0:64, p, qcols],
                            start=True, stop=True, tile_position=(0, 0),
                        )
                        nc.tensor.matmul(
                            st[:, QW + lo:2 * QW],
                            kt[64:128, p, kcols],
                            qt[64:128, p, qcols],
                            start=True, stop=True, tile_position=(64, 0),
                        )
                        pt = work.tile([128, 1024], BF16, tag="pt", bufs=3)
                        nc.scalar.activation(
                            pt[:].rearrange("p (h q) -> p h q", h=2)[:, :, lo:QW],
                            st[:].rearrange("p (h q) -> p h q", h=2)[:, :, lo:QW],
                            AF.Exp,
                        )
                        if o >= 0:
                            # only the leading 128-col strip of the valid range
                            # holds the causal triangle; one strided op covers
                            # both heads' strips (m0 is the triangle, tiled x2)
                            strips = pt[:].rearrange(
                                "p (h q) -> p h q", h=2
                            )[:, :, lo:lo + 128]
                            nc.vector.tensor_mul(
                                strips,
                                strips,
                                m0[:].rearrange("p (h q) -> p h q", h=2),
                            )
                        nc.tensor.matmul(
                            o_A[:, lo:QW],
                            v[:, kb, hA * 65:(hA + 1) * 65],
                            pt[:, lo:QW],
                            start=(kb == 0), stop=(kb == nkb - 1),
                        )
                        nc.tensor.matmul(
                            o_B[:, lo:QW],
                            v[:, kb, hB * 65:(hB + 1) * 65],
                            pt[:, QW + lo:2 * QW],
                            start=(kb == 0), stop=(kb == nkb - 1),
                        )
                        slot += 1
                    # stage o out of PSUM promptly so the o slots free for the
                    # next head-pair (keeps PE from stalling / HAM warm)
                    ocp = work.tile([65, 1024], F32, tag="ocp", bufs=5)
                    nc.vector.tensor_copy(ocp[:, 0:QW], o_A[:])
                    nc.vector.tensor_copy(ocp[:, QW:1024], o_B[:])
                    ocps.append(ocp)
                    if p >= 1:
                        emit_norm(p - 1)
                emit_norm(NP - 1)
                while emitted < len(chunks):
                    chunks[emitted]()
                    emitted += 1

            # last j's output projection (the tail): pipeline two n-blocks so
            # the c=0..2 accumulations run while the final normalize drains
            jrp = slice((NJ - 1) * QW, NJ * QW)
            for npair in range(4):
                yy = []
                for n in (2 * npair, 2 * npair + 1):
                    yps = psum.tile([128, QW], F32, tag="small", bufs=2)
                    for c4 in range(3):
                        nc.tensor.matmul(
                            yps[:],
                            wo[:, c4, n * 128:(n + 1) * 128],
                            ot[:, c4, jrp],
                            start=(c4 == 0), stop=False,
                        )
                    yy.append(yps)
                for i, n in enumerate((2 * npair, 2 * npair + 1)):
                    yps = yy[i]
                    nc.tensor.matmul(
                        yps[:],
                        wo[:, 3, n * 128:(n + 1) * 128],
                        ot[:, 3, jrp],
                        start=False, stop=True,
                    )
                    ysb = work.tile([128, QW], F32, tag="ysb", bufs=3)
                    nc.vector.tensor_copy(ysb[:], yps[:])
                    nc.sync.dma_start(
                        out=yt_d[n * 128:(n + 1) * 128, jrp], in_=ysb[:]
                    )

    nc.finalize()
    return nc


def _prep_inputs(x, Wq, Wk, Wv, Wo, bo):
    """Build the 8 per-core input maps (host-side layout prep only)."""
    import ml_dtypes

    scale = 1.0 / np.sqrt(np.float32(HD))
    kr = np.arange(128, dtype=np.float32)[:, None]
    qc = np.arange(128, dtype=np.float32)[None, :]
    tri = (qc >= kr)
    m0 = np.concatenate([tri, tri], axis=1).astype(ml_dtypes.bfloat16)

    FP8 = ml_dtypes.float8_e4m3  # TRN FP8_EXP4-compatible for |x| <= 240

    def xarr(xb, dtype):  # [T, D] -> [NJ, 128, 8*512], one contiguous DMA per j
        xt = xb.T  # [D, T]
        out = np.stack(
            [
                xt[:, j * QW:(j + 1) * QW]
                .reshape(8, 128, QW).transpose(1, 0, 2).reshape(128, 8 * QW)
                for j in range(NJ)
            ]
        )
        return np.ascontiguousarray(out).astype(dtype)

    xt8s = [xarr(np.clip(x[b], -240, 240), FP8) for b in range(B)]
    xts = [xarr(x[b], ml_dtypes.bfloat16) for b in range(B)]
    in_maps = []
    for c in range(NCORES):
        b, g = c // 2, c % 2
        hs = slice(g * 8, (g + 1) * 8)
        # x128 prescale keeps the small weights inside e4m3's normal range;
        # the kernel multiplies the projection PSUM by 1/128 when casting out.
        # layouts are [128, c*...] so each weight loads as one contiguous DMA
        def warr(wt, dtype):  # [D, 512] -> [128, 8*512], row p = concat_c w[c*128+p]
            return np.ascontiguousarray(
                wt.reshape(8, 128, 512).transpose(1, 0, 2).reshape(128, 8 * 512)
            ).astype(dtype)

        wqc = warr(Wq[hs].reshape(512, D).T * (scale * 128), FP8)
        wkc = warr(Wk[hs].reshape(512, D).T * 128, FP8)
        wvc = warr(Wv[hs].reshape(512, D).T, ml_dtypes.bfloat16)
        woc = np.ascontiguousarray(
            Wo[:, g * 512:(g + 1) * 512].T.reshape(4, 128, D).transpose(1, 0, 2).reshape(128, 4 * D)
        ).astype(ml_dtypes.bfloat16)
        in_maps.append(
            {"xt8": xt8s[b], "xt": xts[b], "wq": wqc, "wk": wkc, "wv": wvc,
             "wo": woc, "mask": m0}
        )
    return in_maps


def _assemble(yts, bo):
    """Sum the per-core partial outputs of each batch pair, add bias."""
    y = np.empty((B, T, D), np.float32)
    for b in range(B):
        y[b] = (yts[2 * b] + yts[2 * b + 1]).T
    y += bo.astype(np.float32)[None, None, :]
    return y


def _run(inputs, trace=False, trace_cores=None):
    from concourse.bass_utils import run_bass_kernel_spmd

    if "nc" not in _CACHE:
        _CACHE["nc"] = _build_nc()
    nc = _CACHE["nc"]
    in_maps = _prep_inputs(
        inputs["x"], inputs["Wq"], inputs["Wk"], inputs["Wv"], inputs["Wo"], inputs["bo"]
    )
    r = run_bass_kernel_spmd(
        nc, in_maps, list(range(NCORES)), trace=trace, trace_cores=trace_cores
    )
    y = _assemble([r.results[c]["yt"] for c in range(NCORES)], inputs["bo"])
    return y, r


def kernel(**inputs):
    y, _ = _run(inputs, trace=False)
    return y


# revision 62
# speedup vs baseline: 1.2598x; 1.0136x over previous
"""Trainium2 Bass kernel for causal multi-head attention (B=4, T=2048, D=1024, H=16).

Sharding: 8 cores = 4 batches x 2 head-groups (8 heads each).
Per core pipeline (Tile framework, single SPMD program):
  phase 1(j): Q/K projections into transposed per-head-pair layout QT/KT [128=2*64, T],
           V projection into [t, 8*65] layout (65th col per head = ones, for rowsums)
  phase 2(j): per (q-range of 512, head-pair): causal flash attention in transposed
           layout: ST[k,q] = KT-slice^T @ QT-slice (row-packed pair of matmuls),
           PT = exp(ST) (ACT), causal triangle mask on diagonal 128-col strips (DVE),
           OT[hd+1, q] += [V|1]^T @ PT (bf16), normalize by approx-reciprocal rowsum.
  phase 3(j): output projection YT[dout, t] = Wo_gT^T @ OT, DMA'd straight from PSUM.
Phase 1(j+1) matmul chunks are emitted interleaved into phase 2(j) so the PE fills
its exp-wait gaps with projection work (phase 2 is ACT-bound; phases 1/3 PE-bound).
No collective: each core emits its partial YT [D, T]; the host adds the two partial
sums of each batch pair and adds the output bias.
"""

import numpy as np

B, T, D, H, HD = 4, 2048, 1024, 16, 64
NCORES = 8
NP = 4          # head pairs per core
NJ = 4          # q-ranges of 512
QW = 512
TB = T // 128   # 16

_CACHE = {}


def _build_nc():
    import concourse.mybir as mybir
    import concourse.tile as tile
    from concourse import bacc

    F32 = mybir.dt.float32
    BF16 = mybir.dt.bfloat16
    FP8 = mybir.dt.float8e4
    AF = mybir.ActivationFunctionType
    DR = mybir.MatmulPerfMode.DoubleRow

    nc = bacc.Bacc(None, target_bir_lowering=False)

    # Pin all activations to the one table holding Exp+Ln+Copy so the
    # act-table chooser can't thrash loads between the exp stream and the
    # exp(-ln(x)) reciprocal. Indices must match act_info.json, so other
    # tables are emptied rather than removed.
    import types as _types
    from concourse.hw_specs import get_activation_tables as _gat

    def _pinned_act_table_loads(self):
        import bass_rust as _bass_rust
        import concourse.mybir as _mybir
        has_activation = any(
            isinstance(i, _mybir.InstActivation)
            for b in self.main_func.blocks
            for i in b.instructions
        )
        if not has_activation:
            return
        tables = [
            (name, funcs if name == "natural_log_exp_and_others" else set())
            for name, funcs in _gat(self.m.arch).items()
        ]
        _bass_rust.insert_act_table_loads(self, tables)

    nc.insert_act_table_loads = _types.MethodType(_pinned_act_table_loads, nc)

    xt8_d = nc.declare_dram_parameter("xt8", [NJ, 128, 8 * QW], FP8, isOutput=False)
    xt_d = nc.declare_dram_parameter("xt", [NJ, 128, 8 * QW], BF16, isOutput=False)
    wq_d = nc.declare_dram_parameter("wq", [128, 8 * 512], FP8, isOutput=False)
    wk_d = nc.declare_dram_parameter("wk", [128, 8 * 512], FP8, isOutput=False)
    wv_d = nc.declare_dram_parameter("wv", [128, 8 * 512], BF16, isOutput=False)
    wo_d = nc.declare_dram_parameter("wo", [128, 4 * D], BF16, isOutput=False)
    mask_d = nc.declare_dram_parameter("mask", [128, 256], BF16, isOutput=False)
    yt_d = nc.declare_dram_parameter("yt", [D, T], F32, isOutput=True)

    with tile.TileContext(nc) as tc:
        with (
            tc.tile_pool(name="persist", bufs=1) as pers,
            tc.tile_pool(name="work", bufs=1) as work,
            tc.tile_pool(name="psum", bufs=1, space="PSUM") as psum,
        ):
            qt = pers.tile([128, NP, T], BF16)
            kt = pers.tile([128, NP, T], BF16)
            v = pers.tile([128, TB, 8 * 65], BF16)
            ot = pers.tile([128, NP, T], BF16)
            m0 = pers.tile([128, 256], BF16)
            wo = pers.tile([128, 4, D], BF16)
            wq = pers.tile([128, 8, 512], FP8)
            wk = pers.tile([128, 8, 512], FP8)
            wv = pers.tile([128, 8, 512], BF16)

            # all per-head ones-columns of V in one strided memset (the V
            # copies never touch column 64, so this never gets overwritten)
            nc.gpsimd.memset(
                v[:].rearrange("p i (h c) -> p i h c", c=65)[:, :, :, 64:65], 1.0
            )

            xs_tiles = {}
            xsb_tiles = {}

            def load_xs(j):
                t = work.tile([128, 8, QW], FP8, tag="xs", bufs=3)
                tb = work.tile([128, 8, QW], BF16, tag="xsb", bufs=3)
                nc.sync.dma_start(
                    out=t[:], in_=xt8_d[j].rearrange("p (c n) -> p c n", c=8)
                )
                nc.sync.dma_start(
                    out=tb[:], in_=xt_d[j].rearrange("p (c n) -> p c n", c=8)
                )
                xs_tiles[j] = t
                xsb_tiles[j] = tb

            # startup DMAs: weights are host-prearranged [128, ...] so each is
            # one fully contiguous transfer per partition
            nc.sync.dma_start(out=m0[:], in_=mask_d[:])
            nc.sync.dma_start(out=wq[:], in_=wq_d.rearrange("p (c n) -> p c n", c=8))
            nc.sync.dma_start(out=wk[:], in_=wk_d.rearrange("p (c n) -> p c n", c=8))
            load_xs(0)
            nc.sync.dma_start(out=wv[:], in_=wv_d.rearrange("p (c n) -> p c n", c=8))
            nc.sync.dma_start(out=wo[:], in_=wo_d.rearrange("p (c n) -> p c n", c=4))

            # Filler work is emitted as ~2-matmul micro-chunks: a whole 8-MM
            # chunk in the PE FIFO delays the next ST by up to ~1.8us, which
            # starves the ACT exp stream (the phase-2 critical path). Each
            # chunk is a pair of closures sharing one PSUM accumulator; the
            # pair stays adjacent in the drain list so at most two
            # accumulation groups are ever open on the "small" tag.
            # Weights are pre-scaled x128 on the host so they sit in e4m3's
            # normal range; the 1/128 is undone in the PSUM->SBUF copy.
            def qk_micros(j, p, w_sb, dst):
                st_ = {}

                def half(h):
                    if h == 0:
                        acc = psum.tile([128, QW], F32, tag="small", bufs=2)
                        st_["acc"] = acc
                    acc = st_["acc"]
                    for cp in (2 * h, 2 * h + 1):
                        nc.tensor.matmul(
                            acc[:],
                            w_sb[:, 2 * cp:2 * cp + 2, p * 128:(p + 1) * 128],
                            xs_tiles[j][:, 2 * cp:2 * cp + 2, :],
                            start=(cp == 0),
                            stop=(cp == 3),
                            perf_mode=DR,
                        )
                    if h == 1:
                        nc.vector.tensor_scalar_mul(
                            dst[:, p, j * QW:(j + 1) * QW], acc[:], 1.0 / 128.0
                        )
                return [lambda: half(0), lambda: half(1)]

            def v_micros(j, sub):
                # V path stays bf16: early tokens average few keys, so v
                # quantization error doesn't wash out like q/k error does
                i = 4 * j + sub
                st_ = {}

                def half(h):
                    if h == 0:
                        acc = psum.tile([128, QW], F32, tag="small", bufs=2)
                        st_["acc"] = acc
                    acc = st_["acc"]
                    for c in range(4 * h, 4 * h + 4):
                        nc.tensor.matmul(
                            acc[:],
                            xsb_tiles[j][:, c, sub * 128:(sub + 1) * 128],
                            wv[:, c, :],
                            start=(c == 0),
                            stop=(c == 7),
                        )
                    if h == 1:
                        vblk = v[:, i, :].rearrange("p (h c) -> p h c", c=65)
                        nc.vector.tensor_copy(
                            vblk[:, :, 0:64],
                            acc[:].rearrange("p (h c) -> p h c", c=64),
                        )
                return [lambda: half(0), lambda: half(1)]

            def phase1_micros(j, v_early=False):
                ch = []
                for p in range(NP):
                    ch += qk_micros(j, p, wq, qt)
                    ch += qk_micros(j, p, wk, kt)
                    if v_early and p == 0:
                        for sub in range(4):
                            ch += v_micros(j, sub)
                if not v_early:
                    for sub in range(4):
                        ch += v_micros(j, sub)
                return ch

            # phase 1(0): only Q0/K0 up front — everything else interleaves
            # into phase 2(0)'s slots so the exp stream starts ~10us earlier
            for m_ in qk_micros(0, 0, wq, qt) + qk_micros(0, 0, wk, kt):
                m_()
            load_xs(1)
            # V blocks land just before their AVs; Q/K(p) just before pair p.
            # Targets are non-decreasing along the list (drain pops in order).
            leftover0 = (
                v_micros(0, 0)
                + qk_micros(0, 1, wq, qt) + v_micros(0, 1)
                + qk_micros(0, 1, wk, kt) + v_micros(0, 2)
                + v_micros(0, 3)
                + qk_micros(0, 2, wq, qt) + qk_micros(0, 2, wk, kt)
                + qk_micros(0, 3, wq, qt) + qk_micros(0, 3, wk, kt)
            )
            leftover0_targets = [0, 0, 1, 1, 1, 1, 2, 2, 2, 2,
                                 3, 3, 4, 4, 5, 5, 8, 8, 9, 9]

            def p3_micros(j, n):
                jrp = slice(j * QW, (j + 1) * QW)
                st_ = {}

                def half(h):
                    if h == 0:
                        yps = psum.tile([128, QW], F32, tag="small", bufs=2)
                        st_["y"] = yps
                    yps = st_["y"]
                    for c4 in (2 * h, 2 * h + 1):
                        nc.tensor.matmul(
                            yps[:],
                            wo[:, c4, n * 128:(n + 1) * 128],
                            ot[:, c4, jrp],
                            start=(c4 == 0), stop=(c4 == 3),
                        )
                    if h == 1:
                        ysb = work.tile([128, QW], F32, tag="ysb", bufs=3)
                        nc.vector.tensor_copy(ysb[:], yps[:])
                        nc.sync.dma_start(
                            out=yt_d[n * 128:(n + 1) * 128, jrp], in_=ysb[:]
                        )
                return [lambda: half(0), lambda: half(1)]

            def phase3_chunks(j):
                ch = []
                for n in range(8):
                    ch += p3_micros(j, n)
                return ch

            carry_kv = []
            for j in range(NJ):
                jr = slice(j * QW, (j + 1) * QW)
                # PE filler work for this j's ACT-bound attention stream:
                # previous j's output projection + next j's projections
                p3 = phase3_chunks(j - 1) if j > 0 else []
                if j + 2 < NJ:
                    load_xs(j + 2)
                nkb = 4 * j + 4
                slots = NP * nkb
                if j + 1 == NJ - 1:
                    # defer the last j's K/V projections into phase 2(3)
                    # itself: only its diagonal blocks (kb>=12) consume them,
                    # and j=3 is the ACT-bound stretch that starves the PE
                    p1 = []
                    for p in range(NP):
                        p1 += qk_micros(j + 1, p, wq, qt)
                    carry_kv = []
                    for p in range(NP):
                        carry_kv += qk_micros(j + 1, p, wk, kt)
                    for s in range(4):
                        carry_kv += v_micros(j + 1, s)
                elif j + 1 < NJ:
                    p1 = phase1_micros(j + 1)
                else:
                    p1 = []
                if j == NJ - 1:
                    # deferred K/V first (needed from kb=12 on), then the
                    # previous j's output projection spread across the rest
                    chunks = carry_kv + p3
                    targets = [1 + (i * 10) // len(carry_kv) for i in range(len(carry_kv))] + [
                        14 + (i * 50) // max(1, len(p3) - 1) for i in range(len(p3))
                    ]
                else:
                    # interleave micro-pairs of phase3(j-1) and phase1(j+1),
                    # keeping each chunk's two halves adjacent
                    chunks = []
                    for i in range(max(len(p3), len(p1)) // 2 + 1):
                        if 2 * i < len(p3):
                            chunks += p3[2 * i:2 * i + 2]
                        if 2 * i < len(p1):
                            chunks += p1[2 * i:2 * i + 2]
                    targets = [
                        (i + 1) * slots // (len(chunks) + 3)
                        for i in range(len(chunks))
                    ]
                    if j == 0:
                        # rest of phase 1(0), placed just ahead of its consumers
                        chunks = leftover0 + chunks
                        targets = leftover0_targets + [
                            max(t, 13) for t in targets
                        ]
                emitted = 0
                slot = 0
                ocps = []

                def emit_norm(p):
                    # 1/r = exp(-ln(r)): Ln/Exp share the pinned act table.
                    # Emitted one head-pair late so the ln never waits on the
                    # ocp copy inside the strict ACT FIFO.
                    lnr = work.tile([1, 1024], F32, tag="lnr", bufs=2)
                    nc.scalar.activation(lnr[:], ocps[p][64:65, :], AF.Ln)
                    rec = work.tile([1, 1024], F32, tag="rec", bufs=4)
                    nc.scalar.activation(rec[:], lnr[:], AF.Exp, scale=-1.0)
                    bc = work.tile([64, 1024], F32, tag="bc", bufs=3)
                    nc.gpsimd.partition_broadcast(bc[:], rec[:], channels=64)
                    nc.vector.tensor_mul(ot[0:64, p, jr], ocps[p][0:64, 0:QW], bc[:, 0:QW])
                    nc.vector.tensor_mul(ot[64:128, p, jr], ocps[p][0:64, QW:1024], bc[:, QW:1024])

                # ---------------- phase 2(j) with phase 1(j+1) interleaved ----------
                for p in range(NP):
                    hA, hB = 2 * p, 2 * p + 1
                    o_A = psum.tile([65, QW], F32, tag="o", bufs=2)
                    o_B = psum.tile([65, QW], F32, tag="o", bufs=2)
                    for kb in range(nkb):
                        # drain filler BEFORE the kb body so producers land
                        # ahead of their phase-2 consumers
                        while emitted < len(chunks) and slot >= targets[emitted]:
                            chunks[emitted]()
                            emitted += 1
                        o = kb - 4 * j  # diagonal offset; < 0 means full block
                        lo = 128 * o if o > 0 else 0
                        st = psum.tile([128, 1024], F32, tag="st", bufs=2)
                        kcols = slice(kb * 128, (kb + 1) * 128)
                        qcols = slice(j * QW + lo, (j + 1) * QW)
                        nc.tensor.matmul(
                            st[:, lo:QW],
                            kt[0:64, p, kcols],
                            qt[0:64, p, qcols],
                            start=True, stop=True, tile_position=(0, 0),
                        )
                        nc.tensor.matmul(
                            st[:, QW + lo:2 * QW],
                            kt[64:128, p, kcols],
                            qt[64:128, p, qcols],
                            start=True, stop=True, tile_position=(64, 0),
                        )
                        pt = work.tile([128, 1024], BF16, tag="pt", bufs=3)
                        nc.scalar.activation(
                            pt[:].rearrange("p (h q) -> p h q", h=2)[:, :, lo:QW],
                            st[:].rearrange("p (h q) -> p h q", h=2)[:, :, lo:QW],
                            AF.Exp,
                        )
                        if o >= 0:
                            # only the leading 128-col strip of the valid range
                            # holds the causal triangle; one strided op covers
                            # both heads' strips (m0 is the triangle, tiled x2)
                            strips = pt[:].rearrange(
                                "p (h q) -> p h q", h=2
                            )[:, :, lo:lo + 128]
                            nc.vector.tensor_mul(
                                strips,
                                strips,
                                m0[:].rearrange("p (h q) -> p h q", h=2),
                            )
                        nc.tensor.matmul(
                            o_A[:, lo:QW],
                            v[:, kb, hA * 65:(hA + 1) * 65],
                            pt[:, lo:QW],
                            start=(kb == 0), stop=(kb == nkb - 1),
                        )
                        nc.tensor.matmul(
                            o_B[:, lo:QW],
                            v[:, kb, hB * 65:(hB + 1) * 65],
                            pt[:, QW + lo:2 * QW],
                            start=(kb == 0), stop=(kb == nkb - 1),
                        )
                        slot += 1
                    # stage o out of PSUM promptly so the o slots free for the
                    # next head-pair (keeps PE from stalling / HAM warm)
                    ocp = work.tile([65, 1024], F32, tag="ocp", bufs=5)
                    nc.vector.tensor_copy(ocp[:, 0:QW], o_A[:])
                    nc.vector.tensor_copy(ocp[:, QW:1024], o_B[:])
                    ocps.append(ocp)
                    if p >= 1:
                        emit_norm(p - 1)
                emit_norm(NP - 1)
                while emitted < len(chunks):
                    chunks[emitted]()
                    emitted += 1

            # last j's output projection (the tail): pipeline two n-blocks so
            # the c=0..2 accumulations run while the final normalize drains
            jrp = slice((NJ - 1) * QW, NJ * QW)
            for npair in range(4):
                yy = []
                for n in (2 * npair, 2 * npair + 1):
                    yps = psum.tile([128, QW], F32, tag="small", bufs=2)
                    for c4 in range(3):
                        nc.tensor.matmul(
                            yps[:],
                            wo[:, c4, n * 128:(n + 1) * 128],
                            ot[:, c4, jrp],
                            start=(c4 == 0), stop=False,
                        )
                    yy.append(yps)
                for i, n in enumerate((2 * npair, 2 * npair + 1)):
                    yps = yy[i]
                    nc.tensor.matmul(
                        yps[:],
                        wo[:, 3, n * 128:(n + 1) * 128],
                        ot[:, 3, jrp],
                        start=False, stop=True,
                    )
                    ysb = work.tile([128, QW], F32, tag="ysb", bufs=3)
                    nc.vector.tensor_copy(ysb[:], yps[:])
                    nc.sync.dma_start(
                        out=yt_d[n * 128:(n + 1) * 128, jrp], in_=ysb[:]
                    )

    nc.finalize()
    return nc


def _prep_inputs(x, Wq, Wk, Wv, Wo, bo):
    """Build the 8 per-core input maps (host-side layout prep only)."""
    import ml_dtypes

    scale = 1.0 / np.sqrt(np.float32(HD))
    kr = np.arange(128, dtype=np.float32)[:, None]
    qc = np.arange(128, dtype=np.float32)[None, :]
    tri = (qc >= kr)
    m0 = np.concatenate([tri, tri], axis=1).astype(ml_dtypes.bfloat16)

    FP8 = ml_dtypes.float8_e4m3  # TRN FP8_EXP4-compatible for |x| <= 240

    def xarr(xb, dtype):  # [T, D] -> [NJ, 128, 8*512], one contiguous DMA per j
        xt = xb.T  # [D, T]
        out = np.stack(
            [
                xt[:, j * QW:(j + 1) * QW]
                .reshape(8, 128, QW).transpose(1, 0, 2).reshape(128, 8 * QW)
                for j in range(NJ)
            ]
        )
        return np.ascontiguousarray(out).astype(dtype)

    xt8s = [xarr(np.clip(x[b], -240, 240), FP8) for b in range(B)]
    xts = [xarr(x[b], ml_dtypes.bfloat16) for b in range(B)]
    in_maps = []
    for c in range(NCORES):
        b, g = c // 2, c % 2
        hs = slice(g * 8, (g + 1) * 8)
        # x128 prescale keeps the small weights inside e4m3's normal range;
        # the kernel multiplies the projection PSUM by 1/128 when casting out.
        # layouts are [128, c*...] so each weight loads as one contiguous DMA
        def warr(wt, dtype):  # [D, 512] -> [128, 8*512], row p = concat_c w[c*128+p]
            return np.ascontiguousarray(
                wt.reshape(8, 128, 512).transpose(1, 0, 2).reshape(128, 8 * 512)
            ).astype(dtype)

        wqc = warr(Wq[hs].reshape(512, D).T * (scale * 128), FP8)
        wkc = warr(Wk[hs].reshape(512, D).T * 128, FP8)
        wvc = warr(Wv[hs].reshape(512, D).T, ml_dtypes.bfloat16)
        woc = np.ascontiguousarray(
            Wo[:, g * 512:(g + 1) * 512].T.reshape(4, 128, D).transpose(1, 0, 2).reshape(128, 4 * D)
        ).astype(ml_dtypes.bfloat16)
        in_maps.append(
            {"xt8": xt8s[b], "xt": xts[b], "wq": wqc, "wk": wkc, "wv": wvc,
             "wo": woc, "mask": m0}
        )
    return in_maps


def _assemble(yts, bo):
    """Sum the per-core partial outputs of each batch pair, add bias."""
    y = np.empty((B, T, D), np.float32)
    for b in range(B):
        y[b] = (yts[2 * b] + yts[2 * b + 1]).T
    y += bo.astype(np.float32)[None, None, :]
    return y


def _run(inputs, trace=False, trace_cores=None):
    from concourse.bass_utils import run_bass_kernel_spmd

    if "nc" not in _CACHE:
        _CACHE["nc"] = _build_nc()
    nc = _CACHE["nc"]
    in_maps = _prep_inputs(
        inputs["x"], inputs["Wq"], inputs["Wk"], inputs["Wv"], inputs["Wo"], inputs["bo"]
    )
    r = run_bass_kernel_spmd(
        nc, in_maps, list(range(NCORES)), trace=trace, trace_cores=trace_cores
    )
    y = _assemble([r.results[c]["yt"] for c in range(NCORES)], inputs["bo"])
    return y, r


def kernel(**inputs):
    y, _ = _run(inputs, trace=False)
    return y
